# revision 54
# baseline (speedup 1.0000x reference)
"""2-layer GAT (graph attention) Bass/Tile kernel for Trainium2, 8-core SPMD.

Sharding: nodes partitioned contiguously across cores; edges assigned to the
core owning their dst, sorted by dst, grouped into 128-dst blocks with
uniformly padded lo/hi slot halves (int16 gather index limit) so all cores
share one SPMD module.

Per core: both layers' node-feature tables are built LOCALLY (each core
transforms only its own nodes: x@[W1|A1] resp. h@[W2|A2], PSUM slices on a
512-f32 pitch so no matmul output crosses a 2KB PSUM bank) and AllGather'ed
into rank-major gather tables — the halo exchange of transformed src node
features.  tab1 rows [feat(d-major) | el | er] bf16 (768B rows for
dma_gather), tab2 rows 512B.  Because both tables share the rank-major id
space, one meta/midx tensor pair serves both edge phases.  Edge phase per
block: one packed int8 meta DMA carries pre-built fp8 scatter/gather
one-hot matrices (PE accepts fp8 lhsT with bf16 rhs, so no per-chunk DVE
is_equal builds), a small separate DMA carries the int16 gather indices;
multi-packet dma_gather fetches src rows into one fused lo+hi tile; PE
matmuls broadcast er to slots and scatter-add messages + exp-sums into
PSUM.  Features use a d-major (d,h) layout so the
attention-weight broadcast is along a non-innermost axis, keeping the DVE
2x mode for the message multiply; leaky-relu runs on DVE (the Lrelu ACT
table lives in a different func set than Exp and would reload every
block), exp/relu on ACT.  Per-dst softmax normalization happens after the
reduction (max-subtraction skipped; |e| is O(1)).

Host precomputes (numpy, index/layout-only plus weight repacking): slot
maps, fp8 one-hots, wrapped int16 gather indices, d-major-permuted
[W | W@al | W@ar] rhs blocks, bias rows, and the bf16 x transpose.
"""

import os

import numpy as np

import concourse.bacc as bacc
import concourse.bass as bass
import concourse.mybir as mybir
import concourse.tile as tile
from concourse.masks import make_identity

F32 = mybir.dt.float32
BF16 = mybir.dt.bfloat16
I32 = mybir.dt.int32
I16 = mybir.dt.int16
I8 = mybir.dt.int8
FP8 = mybir.dt.float8e4
AF = mybir.ActivationFunctionType
OP = mybir.AluOpType

P = 128
HALF_LIMIT = 32768  # int16 gather index limit
ONE_FP8 = 0x38      # float8e4m3 encoding of 1.0


class GATCfg:
    def __init__(self, N=50000, C=8, IN=128, HID=32, HEADS=8, OUT=16, NEG=0.2):
        self.N, self.C, self.IN = N, C, IN
        self.HID, self.HEADS, self.OUT, self.NEG = HID, HEADS, OUT, NEG
        self.F1 = HEADS * HID        # 256
        self.F2 = HEADS * OUT        # 128
        self.FP8T1 = os.environ.get("GAT_FP8T1", "0") == "1"
        # tab1 row in BYTES: [feat | el]; fp8 feat: 256+16 -> 512B rows,
        # bf16 feat: 512+16 -> 768B rows (dma_gather needs 256B multiples)
        self.ROW1B = 512 if self.FP8T1 else 768
        self.T1USED = (256 if self.FP8T1 else 512) + 16
        self.ROW2 = 256              # bf16 elems; 512B rows (used: 128+8+8)
        self.Nloc = (N + C - 1) // C
        self.NB = (self.Nloc + P - 1) // P
        self.Nlp = self.NB * P
        self.NP1 = ((N + 511) // 512) * 512
        self.NP2 = C * self.Nlp
        NPmax = max(self.NP1, self.NP2)
        h = (NPmax // 2 + P - 1) // P * P
        self.HALF = (min(HALF_LIMIT, max(h, NPmax - HALF_LIMIT))
                     if NPmax > HALF_LIMIT else NPmax)
        self.HALF = max(self.HALF, NPmax - HALF_LIMIT)
        self.SLO = 0
        self.SHI = 0

    @property
    def NCt(self):
        return (self.SLO + self.SHI) // P

    def meta_cols(self):
        return 2 * self.NCt * P

    def midx_cols(self):
        return 2 * (self.SLO // 16) + 2 * (self.SHI // 16)


def _wrap16(vals_slots, S):
    """[S] slot-ordered ints -> [128, S//16] 16-wrapped, replicated 8x."""
    a = vals_slots.reshape(S // 16, 16)
    out = np.zeros((128, S // 16), np.int16)
    blkcols = a.T.astype(np.int16)  # [16, S//16]
    for r in range(8):
        out[r * 16:(r + 1) * 16, :] = blkcols
    return out


def prep_indices(src, dst, cfg):
    """Host index-only preprocessing: slot maps, gather indices, fp8 one-hots."""
    C, Nloc, NB = cfg.C, cfg.Nloc, cfg.NB
    src = np.asarray(src).astype(np.int64)
    dst = np.asarray(dst).astype(np.int64)
    core = dst // Nloc
    dloc = dst - core * Nloc
    blk = dloc // P
    dblk = dloc - blk * P
    key = core * NB + blk
    order = np.argsort(key, kind="stable")
    counts = np.bincount(key, minlength=C * NB)
    starts = np.zeros(C * NB + 1, np.int64)
    np.cumsum(counts, out=starts[1:])

    r_of = src // Nloc
    srcp = r_of * cfg.Nlp + (src - r_of * Nloc)   # layer-2 rank-major id

    HALFc = cfg.HALF

    def lohi_max(ids):
        lo_max = hi_max = 0
        for k in range(C * NB):
            e = order[starts[k]:starts[k + 1]]
            n_lo = int((ids[e] < HALFc).sum())
            lo_max = max(lo_max, n_lo)
            hi_max = max(hi_max, e.size - n_lo)
        return lo_max, hi_max

    lo1, hi1 = lohi_max(srcp)
    SLO = ((max(lo1, 1) + P - 1) // P) * P
    SHI = ((max(hi1, 1) + P - 1) // P) * P
    cfg.SLO, cfg.SHI = SLO, SHI
    NCt = cfg.NCt
    CL, CH = SLO // 16, SHI // 16
    MC = cfg.meta_cols()

    out = {}
    MI = 2 * CL + 2 * CH
    for layer, ids in ((1, srcp),):
        meta = np.zeros((C, NB, 128, MC), np.uint8)
        midx = np.zeros((C, NB, 128, MI), np.uint8)
        for c in range(C):
            for b in range(NB):
                k = c * NB + b
                e = order[starts[k]:starts[k + 1]]
                v = ids[e]
                m = v < HALFc
                elo, ehi = e[m], e[~m]
                ilo = np.zeros(SLO, np.int64)
                ihi = np.zeros(SHI, np.int64)
                ilo[:elo.size] = v[m]
                ihi[:ehi.size] = v[~m] - HALFc
                # slot ids: lo at [0,nlo), hi at [SLO, SLO+nhi)
                sl = np.concatenate([np.arange(elo.size),
                                     SLO + np.arange(ehi.size)])
                dv = dblk[np.concatenate([elo, ehi])]
                oh = np.zeros((128, NCt * P), np.uint8)
                ohT = np.zeros((128, NCt * P), np.uint8)
                # oh[p, c*128 + m] = (m == d(slot c*128+p))
                oh[sl % P, (sl // P) * P + dv] = ONE_FP8
                # ohT[p, c*128 + j] = (d(slot c*128+j) == p)
                ohT[dv, (sl // P) * P + sl % P] = ONE_FP8
                wl = _wrap16(ilo, SLO).view(np.uint8).reshape(128, 2 * CL)
                wh = _wrap16(ihi, SHI).view(np.uint8).reshape(128, 2 * CH)
                meta[c, b] = np.concatenate([oh, ohT], axis=1)
                midx[c, b] = np.concatenate([wl, wh], axis=1)
        out[f"meta{layer}"] = np.ascontiguousarray(
            meta.transpose(0, 2, 1, 3).reshape(C, 128, NB * MC))
        out[f"midx{layer}"] = np.ascontiguousarray(
            midx.transpose(0, 2, 1, 3).reshape(C, 128, NB * MI))
    return out


def _dperm(H, D):
    """permutation p with p[d*H+h] = h*D+d (d-major ordering)."""
    idx = np.arange(H * D).reshape(H, D).T.reshape(-1)
    return idx


def host_inputs(inputs, cfg, idx):
    import ml_dtypes
    BF = ml_dtypes.bfloat16
    H, D1, D2 = cfg.HEADS, cfg.HID, cfg.OUT
    F1, F2 = cfg.F1, cfg.F2
    p1 = _dperm(H, D1)   # F1 perm
    p2 = _dperm(H, D2)   # F2 perm

    x = np.asarray(inputs["x"], np.float32)
    xT = np.zeros((cfg.IN, cfg.N), np.float32)
    xT[:, :cfg.N] = np.ascontiguousarray(x.T)

    W1 = np.asarray(inputs["W1"], np.float32)
    W2 = np.asarray(inputs["W2"], np.float32)
    al1 = np.asarray(inputs["al1"], np.float32)
    ar1 = np.asarray(inputs["ar1"], np.float32)
    al2 = np.asarray(inputs["al2"], np.float32)
    ar2 = np.asarray(inputs["ar2"], np.float32)
    b1 = np.asarray(inputs["b1"], np.float32)
    b2 = np.asarray(inputs["b2"], np.float32)

    # el/er projection matrices [IN, 16]: col h = W1[:, head h] @ al1[h]
    A1 = np.zeros((cfg.IN, 16), np.float32)
    for h in range(H):
        A1[:, h] = W1[:, h * D1:(h + 1) * D1] @ al1[h]
        A1[:, 8 + h] = W1[:, h * D1:(h + 1) * D1] @ ar1[h]
    RHS1 = np.concatenate([W1[:, p1], A1], axis=1)          # [IN, 272]

    # layer2: rows of W2 permuted to d-major F1; cols to d-major F2
    W2p = W2[p1][:, p2]                                      # [256, 128]
    A2 = np.zeros((F1, 16), np.float32)
    for h in range(H):
        A2[:, h] = W2[:, h * D2:(h + 1) * D2] @ al2[h]
        A2[:, 8 + h] = W2[:, h * D2:(h + 1) * D2] @ ar2[h]
    A2 = A2[p1]
    RHS2f = np.concatenate([W2p, A2], axis=1)                # [256, 144]
    # pack as [128, 2, 144] (partition = K within half)
    RHS2 = np.ascontiguousarray(
        RHS2f.reshape(2, 128, 144).transpose(1, 0, 2))

    B1 = np.broadcast_to(b1[p1][None, :], (128, F1)).astype(np.float32)
    b2m = b2.reshape(H, D2).mean(axis=0)                     # [16]
    B2M = np.broadcast_to(b2m[None, :], (128, D2)).astype(np.float32)

    in_maps = []
    for c in range(cfg.C):
        xTl = np.zeros((cfg.IN, cfg.Nlp), np.float32)
        lo = c * cfg.Nloc
        hi = min(cfg.N, lo + cfg.Nloc)
        xTl[:, :hi - lo] = xT[:, lo:hi]
        in_maps.append({
            "xTl": xTl.astype(BF),
            "RHS1": RHS1.astype(BF), "RHS2": RHS2.astype(BF),
            "B1": np.ascontiguousarray(B1),
            "B2M": np.ascontiguousarray(B2M),
            "meta1": idx["meta1"][c].view(np.int8),
            "midx1": idx["midx1"][c].view(np.int8),
        })
    return in_maps


def build_module(cfg, dbg=False, skip_cc=False):
    nc = bacc.Bacc("TRN2", target_bir_lowering=False, debug=False,
                   num_devices=cfg.C)
    C, NB, Nlp = cfg.C, cfg.NB, cfg.Nlp
    F1, F2, ROW2 = cfg.F1, cfg.F2, cfg.ROW2
    ROW1E = cfg.ROW1B // 2       # tab1 row in bf16 elems (bf16 mode only)
    assert not cfg.FP8T1
    SLO, SHI, NCt = cfg.SLO, cfg.SHI, cfg.NCt
    NLOC, NHIC = SLO // P, SHI // P
    CL, CH = SLO // 16, SHI // 16
    MC = cfg.meta_cols()
    MI = cfg.midx_cols()
    OUTW = cfg.OUT
    NT1 = cfg.NP1 // P            # 392 layer-1 table tiles
    TB1 = 8                       # tiles per L1 table DMA group
    NG1 = NT1 // TB1              # 49
    TL2 = 7                       # tiles per L2 table load
    NG2 = NB // TL2               # 7

    d_xTl = nc.dram_tensor("xTl", [cfg.IN, Nlp], BF16, kind="ExternalInput")
    d_RHS1 = nc.dram_tensor("RHS1", [cfg.IN, F1 + 16], BF16,
                            kind="ExternalInput")
    d_RHS2 = nc.dram_tensor("RHS2", [P, 2, F2 + 16], BF16,
                            kind="ExternalInput")
    d_B1 = nc.dram_tensor("B1", [P, F1], F32, kind="ExternalInput")
    d_B2M = nc.dram_tensor("B2M", [P, OUTW], F32, kind="ExternalInput")
    d_meta1 = nc.dram_tensor("meta1", [P, NB * MC], I8, kind="ExternalInput")
    d_midx1 = nc.dram_tensor("midx1", [P, NB * MI], I8, kind="ExternalInput")
    d_out = nc.dram_tensor("out", [NB, P, OUTW], F32, kind="ExternalOutput")

    d_t1l = nc.dram_tensor("t1l", [NB, P, ROW1E], BF16, kind="Internal")
    d_tab1 = nc.dram_tensor("tab1", [C, NB, P, ROW1E], BF16, kind="Internal",
                            addr_space="Shared" if C > 4 else "Local")
    d_t2l = nc.dram_tensor("t2l", [NB, P, ROW2], BF16, kind="Internal")
    d_tab2 = nc.dram_tensor("tab2", [C, NB, P, ROW2], BF16, kind="Internal",
                            addr_space="Shared" if C > 4 else "Local")
    d_hT = nc.dram_tensor("hT", [P, NB, 2, P], BF16, kind="Internal")

    tab1_flat = d_tab1.rearrange("r t p c -> (r t p) c")
    tab2_flat = d_tab2.rearrange("r t p c -> (r t p) c")

    with tile.TileContext(nc) as tc:
        with (
            tc.tile_pool(name="const", bufs=1) as cpool,
            tc.tile_pool(name="work", bufs=6) as wpool,
            tc.tile_pool(name="gath", bufs=2) as gpool,
            tc.tile_pool(name="stage", bufs=4) as spool,
            tc.tile_pool(name="idx", bufs=8) as ipool,
        ):
            # ---------------- constants ----------------
            ident = cpool.tile([P, P], BF16)
            make_identity(nc, ident[:])
            rhs1 = cpool.tile([P, F1 + 16], BF16)
            nc.sync.dma_start(rhs1[:], d_RHS1[:, :])
            rhs2 = cpool.tile([P, 2, F2 + 16], BF16)
            nc.sync.dma_start(rhs2[:], d_RHS2[:, :, :])
            b1row = cpool.tile([P, F1], F32)
            nc.sync.dma_start(b1row[:], d_B1[:, :])
            b2mean = cpool.tile([P, OUTW], F32)
            nc.sync.dma_start(b2mean[:], d_B2M[:, :])

            # ------- local layer-1 table (rows [feat|el|er]) + allgather ---
            with tc.tile_pool(name="t1ps", bufs=2, space="PSUM") as t1ps:
                for g in range(NG2):
                    xl = wpool.tile([P, TL2 * P], BF16, tag="xl")
                    nc.sync.dma_start(
                        xl[:], d_xTl[:, g * TL2 * P:(g + 1) * TL2 * P])
                    stg = spool.tile([P, TL2, F1 + 16], BF16, tag="stg1")
                    for q0, qn in ((0, 2), (2, 2), (4, 2), (6, 1)):
                        # 512-f32 slice pitch keeps each matmul output inside
                        # a single 2KB PSUM bank (outputs must not cross one)
                        ps = t1ps.tile([P, 2, 512], F32, tag="t1")
                        for m in range(qn):
                            nc.tensor.matmul(
                                ps[:, m, 0:F1 + 16],
                                lhsT=xl[:, (q0 + m) * P:(q0 + m + 1) * P],
                                rhs=rhs1[:], start=True, stop=True)
                        (nc.vector.tensor_copy if q0 % 4 == 0
                         else nc.scalar.copy)(
                            stg[:, q0:q0 + qn, :],
                            ps[:, 0:qn, 0:F1 + 16])
                    nc.sync.dma_start(
                        d_t1l[g * TL2:(g + 1) * TL2, :, 0:F1 + 16]
                        .rearrange("t p c -> p t c"), stg[:])

            if C > 1 and not skip_cc:
                nc.gpsimd.collective_compute(
                    "AllGather", OP.bypass,
                    replica_groups=[list(range(C))],
                    ins=[d_t1l[:, :, :]],
                    outs=[d_tab1[:, :, :, :]],
                )

            # ---------------- edge phase (shared) ----------------
            def edge_phase(layer, pspool, tps):
                F = F1 if layer == 1 else F2
                ROW = ROW1E if layer == 1 else ROW2
                tab = tab1_flat if layer == 1 else tab2_flat
                d_meta, d_midx = d_meta1, d_midx1
                PIECE = 768
                single_packet = "mp" not in os.environ.get("GAT_OPT", "mp")
                for b in range(NB):
                    midx = ipool.tile([P, MI], I8, tag="midx")
                    nc.sync.dma_start(midx[:], d_midx[:, b * MI:(b + 1) * MI])
                    meta = wpool.tile([P, MC], I8, tag="meta")
                    nc.sync.dma_start(meta[:], d_meta[:, b * MC:(b + 1) * MC])
                    il = midx[:, 0:2 * CL].bitcast(I16)
                    ih = midx[:, 2 * CL:MI].bitcast(I16)
                    erblk = ipool.tile([P, 8], BF16, tag="erblk")
                    if layer == 1:
                        nc.sync.dma_start(erblk[:],
                                          d_t1l[b, :, F1 + 8:F1 + 16])
                    else:
                        nc.sync.dma_start(erblk[:],
                                          d_t2l[b, :, F2 + 8:F2 + 16])
                    G = gpool.tile([P, NCt, ROW], BF16, tag=f"G{layer}")
                    Gfeat = G[:, :, 0:F]
                    Gel = G[:, :, F:F + 8]
                    if single_packet:
                        pieces = [(s0, min(PIECE, SLO - s0), 0)
                                  for s0 in range(0, SLO, PIECE)]
                        pieces += [(s0, min(PIECE, SHI - s0), 1)
                                   for s0 in range(0, SHI, PIECE)]
                    else:
                        pieces = [(0, SLO, 0), (0, SHI, 1)]
                    for s0, n, is_hi in pieces:
                        idxs = ih if is_hi else il
                        base = SLO if is_hi else 0
                        src_ap = tab[cfg.HALF:, :] if is_hi else tab[:, :]
                        nc.gpsimd.dma_gather(
                            out_ap=G[:, (base + s0) // P:(base + s0 + n) // P, :],
                            in_ap=src_ap,
                            idxs_ap=idxs[:, s0 // 16:(s0 + n) // 16],
                            num_idxs=n, num_idxs_reg=n, elem_size=ROW,
                            single_packet=single_packet)

                    # er broadcast to slots: fp8 one-hot lhsT
                    erps = tps.tile([P, NCt, 8], F32, tag="erps")
                    for c in range(NCt):
                        nc.tensor.matmul(
                            erps[:, c, :],
                            lhsT=meta[:, (NCt + c) * P:(NCt + c + 1) * P]
                            .bitcast(FP8),
                            rhs=erblk[:], start=True, stop=True)
                    e_t = wpool.tile([P, NCt, 8], BF16, tag="e_t")
                    nc.vector.tensor_tensor(out=e_t[:], in0=Gel,
                                            in1=erps[:], op=OP.add)
                    # leaky_relu on DVE: es = max(e, 0.2*e)  (the Lrelu ACT
                    # table lives in a different func set than Exp; switching
                    # would reload the 1283ns act table every block)
                    ea = wpool.tile([P, NCt, 8], BF16, tag="ea")
                    nc.vector.tensor_scalar(ea[:], e_t[:], cfg.NEG, None,
                                            op0=OP.mult)
                    es = wpool.tile([P, NCt, 8], BF16, tag="es")
                    nc.vector.tensor_tensor(out=es[:], in0=e_t[:], in1=ea[:],
                                            op=OP.max)
                    MSG = gpool.tile([P, NCt, F + 8], BF16, tag=f"MSG{layer}")
                    nc.scalar.activation(MSG[:, :, F:F + 8], es[:], AF.Exp)
                    nc.vector.tensor_tensor(
                        out=MSG[:, :, 0:F].rearrange(
                            "p c (d h) -> p c d h", h=8),
                        in0=Gfeat.rearrange("p c (d h) -> p c d h", h=8),
                        in1=MSG[:, :, F:F + 8].rearrange(
                            "p c (one h) -> p c one h", one=1)
                        .to_broadcast([P, NCt, F // 8, 8]),
                        op=OP.mult)
                    ps = pspool.tile([P, F + 8], F32, tag="eps")
                    for c in range(NCt):
                        nc.tensor.matmul(
                            ps[:],
                            lhsT=meta[:, c * P:(c + 1) * P].bitcast(FP8),
                            rhs=MSG[:, c, :],
                            start=(c == 0), stop=(c == NCt - 1))
                    esum = wpool.tile([P, 8], F32, tag="esum")
                    nc.vector.tensor_scalar(esum[:], ps[:, F:F + 8], 1e-30,
                                            None, op0=OP.max)
                    inv = wpool.tile([P, 8], F32, tag="inv")
                    nc.vector.reciprocal(inv[:], esum[:])
                    yield b, ps, inv

            # ---------------- layer-1 edges + hT + local tab2 ----------------
            with tc.tile_pool(name="e1ps", bufs=2, space="PSUM") as e1ps, \
                 tc.tile_pool(name="tps", bufs=3, space="PSUM") as tps:
                for b, ps, inv in edge_phase(1, e1ps, tps):
                    z = wpool.tile([P, F1], F32, tag="z")
                    nc.vector.tensor_tensor(
                        out=z[:].rearrange("p (d h) -> p d h", h=8),
                        in0=ps[:, 0:F1].rearrange("p (d h) -> p d h", h=8),
                        in1=inv[:].rearrange("p (one h) -> p one h", one=1)
                        .to_broadcast([P, F1 // 8, 8]),
                        op=OP.mult)
                    nc.vector.tensor_add(z[:], z[:], b1row[:])
                    # elu(z) = relu(z) + exp(min(z,0)) - 1
                    # min(z,0) = -relu(-z); both relu and exp run on ACT
                    zmn = wpool.tile([P, F1], BF16, tag="zmn")
                    nc.scalar.activation(zmn[:], z[:], AF.Relu, scale=-1.0)
                    zp = wpool.tile([P, F1], BF16, tag="zp")
                    nc.scalar.activation(zp[:], z[:], AF.Relu)
                    q_ = wpool.tile([P, F1], BF16, tag="q_")
                    nc.scalar.activation(q_[:], zmn[:], AF.Exp, scale=-1.0)
                    hb = wpool.tile([P, F1], BF16, tag="hb")
                    nc.vector.affine_then_add(hb[:], q_[:], zp[:], 1.0, -1.0)
                    pst = tps.tile([P, 2, P], BF16, tag="pst")
                    for q in range(2):
                        nc.tensor.transpose(pst[:, q, :],
                                            hb[:, q * P:(q + 1) * P], ident[:])
                    htp = wpool.tile([P, 2, P], BF16, tag="htp")
                    nc.vector.tensor_copy(htp[:], pst[:])
                    nc.sync.dma_start(d_hT[:, b, :, :], htp[:])

            # ---------------- local layer-2 table + allgather ----------------
            with tc.tile_pool(name="t2ps", bufs=2, space="PSUM") as t2ps:
                for g in range(NG2):
                    ht = wpool.tile([P, TL2, 2, P], BF16, tag="ht")
                    nc.sync.dma_start(ht[:], d_hT[:, g * TL2:(g + 1) * TL2,
                                                  :, :])
                    # 256-f32 slice pitch: every 144-wide output stays inside
                    # one 2KB PSUM bank
                    ps2 = t2ps.tile([P, TL2, 256], F32, tag="t2")
                    for m in range(TL2):
                        nc.tensor.matmul(ps2[:, m, 0:F2 + 16],
                                         lhsT=ht[:, m, 0, :],
                                         rhs=rhs2[:, 0, :],
                                         start=True, stop=False)
                        nc.tensor.matmul(ps2[:, m, 0:F2 + 16],
                                         lhsT=ht[:, m, 1, :],
                                         rhs=rhs2[:, 1, :],
                                         start=False, stop=True)
                    st2 = spool.tile([P, TL2, F2 + 16], BF16, tag="stg2")
                    (nc.vector.tensor_copy if g % 2 == 0 else nc.scalar.copy)(
                        st2[:], ps2[:, :, 0:F2 + 16])
                    nc.sync.dma_start(
                        d_t2l[g * TL2:(g + 1) * TL2, :, 0:F2 + 16]
                        .rearrange("t p c -> p t c"), st2[:])

            if C > 1 and not skip_cc:
                nc.gpsimd.collective_compute(
                    "AllGather", OP.bypass,
                    replica_groups=[list(range(C))],
                    ins=[d_t2l[:, :, :]],
                    outs=[d_tab2[:, :, :, :]],
                )

            # ---------------- layer-2 edges + output ----------------
            with tc.tile_pool(name="e2ps", bufs=2, space="PSUM") as e2ps, \
                 tc.tile_pool(name="tps2", bufs=3, space="PSUM") as tps2:
                ostage = None
                for b, ps, inv in edge_phase(2, e2ps, tps2):
                    if b % 8 == 0:
                        ostage = spool.tile([P, 8, OUTW], F32, tag="ostage")
                    inv8 = wpool.tile([P, 8], F32, tag="inv8")
                    nc.vector.tensor_scalar(inv8[:], inv[:], 0.125, None,
                                            op0=OP.mult)
                    w_ = wpool.tile([P, OUTW, 8], F32, tag="w_")
                    nc.vector.tensor_tensor(
                        out=w_[:],
                        in0=ps[:, 0:F2].rearrange("p (d h) -> p d h", h=8),
                        in1=inv8[:].rearrange("p (one h) -> p one h", one=1)
                        .to_broadcast([P, OUTW, 8]),
                        op=OP.mult)
                    s1 = wpool.tile([P, OUTW, 4], F32, tag="s1")
                    nc.vector.tensor_add(s1[:], w_[:, :, 0:4], w_[:, :, 4:8])
                    s2 = wpool.tile([P, OUTW, 2], F32, tag="s2")
                    nc.vector.tensor_add(s2[:], s1[:, :, 0:2], s1[:, :, 2:4])
                    s3 = wpool.tile([P, OUTW], F32, tag="s3")
                    nc.vector.tensor_add(s3[:], s2[:, :, 0], s2[:, :, 1])
                    nc.vector.tensor_add(ostage[:, b % 8, :], s3[:],
                                         b2mean[:])
                    if b % 8 == 7 or b == NB - 1:
                        b0 = (b // 8) * 8
                        nt = b - b0 + 1
                        nc.sync.dma_start(
                            d_out[b0:b0 + nt, :, :]
                            .rearrange("t p c -> p t c"),
                            ostage[:, 0:nt, :])

            if dbg:
                for nm, src_t in [("dbg_t1l", d_t1l), ("dbg_tab1", d_tab1),
                                  ("dbg_hT", d_hT), ("dbg_t2l", d_t2l),
                                  ("dbg_tab2", d_tab2)]:
                    dd = nc.dram_tensor(nm, list(src_t.shape), BF16,
                                        kind="ExternalOutput")
                    sl = tuple(slice(None) for _ in src_t.shape)
                    nc.sync.dma_start(dd[sl], src_t[sl])

    nc.compile()
    return nc


# ----------------------------------------------------------------------------
_CACHE = {}


def get_built(src, dst, C=8, cfg=None):
    key = (hash(src.tobytes()), hash(dst.tobytes()), C)
    if key not in _CACHE:
        if cfg is None:
            cfg = GATCfg(C=C)
        idx = prep_indices(src, dst, cfg)
        nc = build_module(cfg)
        _CACHE[key] = (cfg, idx, nc)
    return _CACHE[key]


_EXECC = {}


def _get_exec(key, nc, n_cores):
    """Persistent jit(shard_map(bass_exec)) so repeated kernel() calls skip
    retracing/recompiling."""
    if key in _EXECC:
        return _EXECC[key]
    import jax
    from jax.experimental.shard_map import shard_map
    from jax.sharding import Mesh, NamedSharding, PartitionSpec
    from concourse import bass2jax
    bass2jax.install_neuronx_cc_hook()
    partition_name = (nc.partition_id_tensor.name
                      if nc.partition_id_tensor else None)
    in_names, out_names, out_avals, zero_shapes = [], [], [], []
    for alloc in nc.m.functions[0].allocations:
        if not isinstance(alloc, mybir.MemoryLocationSet):
            continue
        name = alloc.memorylocations[0].name
        if alloc.kind == "ExternalInput":
            if name != partition_name:
                in_names.append(name)
        elif alloc.kind == "ExternalOutput":
            out_names.append(name)
            shape = tuple(alloc.tensor_shape)
            dtype = mybir.dt.np(alloc.dtype)
            out_avals.append(jax.core.ShapedArray(shape, dtype))
            zero_shapes.append((shape, dtype))
    n_params = len(in_names)
    in_names_all = list(in_names) + out_names + (
        [partition_name] if partition_name else [])

    def _body(*args):
        ops = list(args)
        if partition_name:
            ops.append(bass2jax.partition_id_tensor())
        outs = bass2jax._bass_exec_p.bind(
            *ops, out_avals=tuple(out_avals), in_names=tuple(in_names_all),
            out_names=tuple(out_names), lowering_input_output_aliases=(),
            sim_require_finite=True, sim_require_nnan=True, nc=nc)
        return tuple(outs)

    devices = jax.devices()[:n_cores]
    mesh = Mesh(np.asarray(devices), ("core",))
    nout = len(out_names)
    f = jax.jit(shard_map(
        _body, mesh=mesh,
        in_specs=(PartitionSpec("core"),) * (n_params + nout),
        out_specs=(PartitionSpec("core"),) * nout, check_rep=False),
        keep_unused=True)
    sh = NamedSharding(mesh, PartitionSpec("core"))
    ent = dict(f=f, in_names=in_names, out_names=out_names,
               zero_shapes=zero_shapes, sh=sh, argcache=None)
    _EXECC[key] = ent
    return ent


def kernel(**inputs) -> np.ndarray:
    import jax
    src = np.asarray(inputs["src"], np.int32)
    dst = np.asarray(inputs["dst"], np.int32)
    x = np.asarray(inputs["x"])
    base = GATCfg(N=int(x.shape[0]), C=8, IN=int(x.shape[1]))
    cfg, idx, nc = get_built(src, dst, C=8, cfg=base)
    in_maps = host_inputs(inputs, cfg, idx)
    key = (hash(src.tobytes()), hash(dst.tobytes()), cfg.C)
    ent = _get_exec(key, nc, cfg.C)
    C = cfg.C
    concat_in = [np.ascontiguousarray(
        np.concatenate([in_maps[c][nm] for c in range(C)], axis=0))
        for nm in ent["in_names"]]
    hashes = tuple(hash(a.tobytes()) for a in concat_in)
    if ent["argcache"] is None or ent["argcache"][0] != hashes:
        zeros = [np.zeros((C * sh0[0], *sh0[1:]), dt)
                 for sh0, dt in ent["zero_shapes"]]
        args = [jax.device_put(a, ent["sh"]) for a in concat_in + zeros]
        ent["argcache"] = (hashes, args)
    args = ent["argcache"][1]
    outs = ent["f"](*args)
    jax.block_until_ready(outs)
    oi = ent["out_names"].index("out")
    out = np.asarray(outs[oi]).reshape(C, cfg.Nlp, cfg.OUT)
    full = out[:, :cfg.Nloc, :].reshape(-1, cfg.OUT)[:cfg.N]
    return np.ascontiguousarray(full.astype(np.float32))


# revision 57
# speedup vs baseline: 1.0048x; 1.0048x over previous
"""2-layer GAT (graph attention) Bass/Tile kernel for Trainium2, 8-core SPMD.

Sharding: nodes partitioned contiguously across cores; edges assigned to the
core owning their dst, sorted by dst, grouped into 128-dst blocks with
uniformly padded lo/hi slot halves (int16 gather index limit) so all cores
share one SPMD module.

Per core: both layers' node-feature tables are built LOCALLY (each core
transforms only its own nodes: x@[W1|A1] resp. h@[W2|A2], PSUM slices on a
512-f32 pitch so no matmul output crosses a 2KB PSUM bank) and AllGather'ed
into rank-major gather tables — the halo exchange of transformed src node
features.  tab1 rows [feat(d-major) | el | er] bf16 (768B rows for
dma_gather), tab2 rows 512B.  Because both tables share the rank-major id
space, one meta/midx tensor pair serves both edge phases.  Edge phase per
block: one packed int8 meta DMA carries pre-built fp8 scatter/gather
one-hot matrices (PE accepts fp8 lhsT with bf16 rhs, so no per-chunk DVE
is_equal builds), a small separate DMA carries the int16 gather indices;
multi-packet dma_gather fetches src rows into one fused lo+hi tile; PE
matmuls broadcast er to slots and scatter-add messages + exp-sums into
PSUM.  Features use a d-major (d,h) layout so the
attention-weight broadcast is along a non-innermost axis, keeping the DVE
2x mode for the message multiply; leaky-relu runs on DVE (the Lrelu ACT
table lives in a different func set than Exp and would reload every
block), exp/relu on ACT.  Per-dst softmax normalization happens after the
reduction (max-subtraction skipped; |e| is O(1)).

Host precomputes (numpy, index/layout-only plus weight repacking): slot
maps, fp8 one-hots, wrapped int16 gather indices, d-major-permuted
[W | W@al | W@ar] rhs blocks, bias rows, and the bf16 x transpose.
"""

import os

import numpy as np

import concourse.bacc as bacc
import concourse.bass as bass
import concourse.mybir as mybir
import concourse.tile as tile
from concourse.masks import make_identity

F32 = mybir.dt.float32
BF16 = mybir.dt.bfloat16
I32 = mybir.dt.int32
I16 = mybir.dt.int16
I8 = mybir.dt.int8
FP8 = mybir.dt.float8e4
AF = mybir.ActivationFunctionType
OP = mybir.AluOpType

P = 128
HALF_LIMIT = 32768  # int16 gather index limit
ONE_FP8 = 0x38      # float8e4m3 encoding of 1.0


class GATCfg:
    def __init__(self, N=50000, C=8, IN=128, HID=32, HEADS=8, OUT=16, NEG=0.2):
        self.N, self.C, self.IN = N, C, IN
        self.HID, self.HEADS, self.OUT, self.NEG = HID, HEADS, OUT, NEG
        self.F1 = HEADS * HID        # 256
        self.F2 = HEADS * OUT        # 128
        self.FP8T1 = os.environ.get("GAT_FP8T1", "0") == "1"
        # tab1 row in BYTES: [feat | el]; fp8 feat: 256+16 -> 512B rows,
        # bf16 feat: 512+16 -> 768B rows (dma_gather needs 256B multiples)
        self.ROW1B = 512 if self.FP8T1 else 768
        self.T1USED = (256 if self.FP8T1 else 512) + 16
        self.ROW2 = 256              # bf16 elems; 512B rows (used: 128+8+8)
        self.Nloc = (N + C - 1) // C
        self.NB = (self.Nloc + P - 1) // P
        self.Nlp = self.NB * P
        self.NP1 = ((N + 511) // 512) * 512
        self.NP2 = C * self.Nlp
        NPmax = max(self.NP1, self.NP2)
        h = (NPmax // 2 + P - 1) // P * P
        self.HALF = (min(HALF_LIMIT, max(h, NPmax - HALF_LIMIT))
                     if NPmax > HALF_LIMIT else NPmax)
        self.HALF = max(self.HALF, NPmax - HALF_LIMIT)
        self.SLO = 0
        self.SHI = 0

    @property
    def NCt(self):
        return (self.SLO + self.SHI) // P

    def meta_cols(self):
        return self.NCt * P

    def midx_cols(self):
        return 2 * (self.SLO // 16) + 2 * (self.SHI // 16)


def _wrap16(vals_slots, S):
    """[S] slot-ordered ints -> [128, S//16] 16-wrapped, replicated 8x."""
    a = vals_slots.reshape(S // 16, 16)
    out = np.zeros((128, S // 16), np.int16)
    blkcols = a.T.astype(np.int16)  # [16, S//16]
    for r in range(8):
        out[r * 16:(r + 1) * 16, :] = blkcols
    return out


def prep_indices(src, dst, cfg):
    """Host index-only preprocessing: slot maps, gather indices, fp8 one-hots."""
    C, Nloc, NB = cfg.C, cfg.Nloc, cfg.NB
    src = np.asarray(src).astype(np.int64)
    dst = np.asarray(dst).astype(np.int64)
    core = dst // Nloc
    dloc = dst - core * Nloc
    blk = dloc // P
    dblk = dloc - blk * P
    key = core * NB + blk
    order = np.argsort(key, kind="stable")
    counts = np.bincount(key, minlength=C * NB)
    starts = np.zeros(C * NB + 1, np.int64)
    np.cumsum(counts, out=starts[1:])

    r_of = src // Nloc
    srcp = r_of * cfg.Nlp + (src - r_of * Nloc)   # layer-2 rank-major id

    HALFc = cfg.HALF

    def lohi_max(ids):
        lo_max = hi_max = 0
        for k in range(C * NB):
            e = order[starts[k]:starts[k + 1]]
            n_lo = int((ids[e] < HALFc).sum())
            lo_max = max(lo_max, n_lo)
            hi_max = max(hi_max, e.size - n_lo)
        return lo_max, hi_max

    lo1, hi1 = lohi_max(srcp)
    SLO = ((max(lo1, 1) + P - 1) // P) * P
    SHI = ((max(hi1, 1) + P - 1) // P) * P
    cfg.SLO, cfg.SHI = SLO, SHI
    NCt = cfg.NCt
    CL, CH = SLO // 16, SHI // 16
    MC = cfg.meta_cols()

    out = {}
    MI = 2 * CL + 2 * CH
    for layer, ids in ((1, srcp),):
        meta = np.zeros((C, NB, 128, MC), np.uint8)
        midx = np.zeros((C, NB, 128, MI), np.uint8)
        for c in range(C):
            for b in range(NB):
                k = c * NB + b
                e = order[starts[k]:starts[k + 1]]
                v = ids[e]
                m = v < HALFc
                elo, ehi = e[m], e[~m]
                ilo = np.zeros(SLO, np.int64)
                ihi = np.zeros(SHI, np.int64)
                ilo[:elo.size] = v[m]
                ihi[:ehi.size] = v[~m] - HALFc
                # slot ids: lo at [0,nlo), hi at [SLO, SLO+nhi)
                sl = np.concatenate([np.arange(elo.size),
                                     SLO + np.arange(ehi.size)])
                dv = dblk[np.concatenate([elo, ehi])]
                oh = np.zeros((128, NCt * P), np.uint8)
                # oh[p, c*128 + m] = (m == d(slot c*128+p))
                oh[sl % P, (sl // P) * P + dv] = ONE_FP8
                wl = _wrap16(ilo, SLO).view(np.uint8).reshape(128, 2 * CL)
                wh = _wrap16(ihi, SHI).view(np.uint8).reshape(128, 2 * CH)
                meta[c, b] = oh
                midx[c, b] = np.concatenate([wl, wh], axis=1)
        out[f"meta{layer}"] = np.ascontiguousarray(
            meta.transpose(0, 2, 1, 3).reshape(C, 128, NB * MC))
        out[f"midx{layer}"] = np.ascontiguousarray(
            midx.transpose(0, 2, 1, 3).reshape(C, 128, NB * MI))
    return out


def _dperm(H, D):
    """permutation p with p[d*H+h] = h*D+d (d-major ordering)."""
    idx = np.arange(H * D).reshape(H, D).T.reshape(-1)
    return idx


def host_inputs(inputs, cfg, idx):
    import ml_dtypes
    BF = ml_dtypes.bfloat16
    H, D1, D2 = cfg.HEADS, cfg.HID, cfg.OUT
    F1, F2 = cfg.F1, cfg.F2
    p1 = _dperm(H, D1)   # F1 perm
    p2 = _dperm(H, D2)   # F2 perm

    x = np.asarray(inputs["x"], np.float32)
    xT = np.zeros((cfg.IN, cfg.N), np.float32)
    xT[:, :cfg.N] = np.ascontiguousarray(x.T)

    W1 = np.asarray(inputs["W1"], np.float32)
    W2 = np.asarray(inputs["W2"], np.float32)
    al1 = np.asarray(inputs["al1"], np.float32)
    ar1 = np.asarray(inputs["ar1"], np.float32)
    al2 = np.asarray(inputs["al2"], np.float32)
    ar2 = np.asarray(inputs["ar2"], np.float32)
    b1 = np.asarray(inputs["b1"], np.float32)
    b2 = np.asarray(inputs["b2"], np.float32)

    # el/er projection matrices [IN, 16]: col h = W1[:, head h] @ al1[h]
    A1 = np.zeros((cfg.IN, 16), np.float32)
    for h in range(H):
        A1[:, h] = W1[:, h * D1:(h + 1) * D1] @ al1[h]
        A1[:, 8 + h] = W1[:, h * D1:(h + 1) * D1] @ ar1[h]
    RHS1 = np.concatenate([W1[:, p1], A1], axis=1)          # [IN, 272]

    # layer2: rows of W2 permuted to d-major F1; cols to d-major F2
    W2p = W2[p1][:, p2]                                      # [256, 128]
    A2 = np.zeros((F1, 16), np.float32)
    for h in range(H):
        A2[:, h] = W2[:, h * D2:(h + 1) * D2] @ al2[h]
        A2[:, 8 + h] = W2[:, h * D2:(h + 1) * D2] @ ar2[h]
    A2 = A2[p1]
    RHS2f = np.concatenate([W2p, A2], axis=1)                # [256, 144]
    # pack as [128, 2, 144] (partition = K within half)
    RHS2 = np.ascontiguousarray(
        RHS2f.reshape(2, 128, 144).transpose(1, 0, 2))

    B1 = np.broadcast_to(b1[p1][None, :], (128, F1)).astype(np.float32)
    b2m = b2.reshape(H, D2).mean(axis=0)                     # [16]
    B2M = np.broadcast_to(b2m[None, :], (128, D2)).astype(np.float32)

    in_maps = []
    for c in range(cfg.C):
        xTl = np.zeros((cfg.IN, cfg.Nlp), np.float32)
        lo = c * cfg.Nloc
        hi = min(cfg.N, lo + cfg.Nloc)
        xTl[:, :hi - lo] = xT[:, lo:hi]
        in_maps.append({
            "xTl": xTl.astype(BF),
            "RHS1": RHS1.astype(BF), "RHS2": RHS2.astype(BF),
            "B1": np.ascontiguousarray(B1),
            "B2M": np.ascontiguousarray(B2M),
            "meta1": idx["meta1"][c].view(np.int8),
            "midx1": idx["midx1"][c].view(np.int8),
        })
    return in_maps


def build_module(cfg, dbg=False, skip_cc=False):
    nc = bacc.Bacc("TRN2", target_bir_lowering=False, debug=False,
                   num_devices=cfg.C)
    C, NB, Nlp = cfg.C, cfg.NB, cfg.Nlp
    F1, F2, ROW2 = cfg.F1, cfg.F2, cfg.ROW2
    ROW1E = cfg.ROW1B // 2       # tab1 row in bf16 elems (bf16 mode only)
    assert not cfg.FP8T1
    SLO, SHI, NCt = cfg.SLO, cfg.SHI, cfg.NCt
    NLOC, NHIC = SLO // P, SHI // P
    CL, CH = SLO // 16, SHI // 16
    MC = cfg.meta_cols()
    MI = cfg.midx_cols()
    OUTW = cfg.OUT
    NT1 = cfg.NP1 // P            # 392 layer-1 table tiles
    TB1 = 8                       # tiles per L1 table DMA group
    NG1 = NT1 // TB1              # 49
    TL2 = 7                       # tiles per L2 table load
    NG2 = NB // TL2               # 7

    d_xTl = nc.dram_tensor("xTl", [cfg.IN, Nlp], BF16, kind="ExternalInput")
    d_RHS1 = nc.dram_tensor("RHS1", [cfg.IN, F1 + 16], BF16,
                            kind="ExternalInput")
    d_RHS2 = nc.dram_tensor("RHS2", [P, 2, F2 + 16], BF16,
                            kind="ExternalInput")
    d_B1 = nc.dram_tensor("B1", [P, F1], F32, kind="ExternalInput")
    d_B2M = nc.dram_tensor("B2M", [P, OUTW], F32, kind="ExternalInput")
    d_meta1 = nc.dram_tensor("meta1", [P, NB * MC], I8, kind="ExternalInput")
    d_midx1 = nc.dram_tensor("midx1", [P, NB * MI], I8, kind="ExternalInput")
    d_out = nc.dram_tensor("out", [NB, P, OUTW], F32, kind="ExternalOutput")

    d_t1l = nc.dram_tensor("t1l", [NB, P, ROW1E], BF16, kind="Internal")
    d_tab1 = nc.dram_tensor("tab1", [C, NB, P, ROW1E], BF16, kind="Internal",
                            addr_space="Shared" if C > 4 else "Local")
    d_t2l = nc.dram_tensor("t2l", [NB, P, ROW2], BF16, kind="Internal")
    d_tab2 = nc.dram_tensor("tab2", [C, NB, P, ROW2], BF16, kind="Internal",
                            addr_space="Shared" if C > 4 else "Local")
    d_hT = nc.dram_tensor("hT", [P, NB, 2, P], BF16, kind="Internal")

    tab1_flat = d_tab1.rearrange("r t p c -> (r t p) c")
    tab2_flat = d_tab2.rearrange("r t p c -> (r t p) c")

    with tile.TileContext(nc) as tc:
        with (
            tc.tile_pool(name="const", bufs=1) as cpool,
            tc.tile_pool(name="work", bufs=6) as wpool,
            tc.tile_pool(name="gath", bufs=2) as gpool,
            tc.tile_pool(name="stage", bufs=4) as spool,
            tc.tile_pool(name="idx", bufs=8) as ipool,
            tc.tile_pool(name="oht", bufs=4) as opool,
        ):
            # ---------------- constants ----------------
            ident = cpool.tile([P, P], BF16)
            make_identity(nc, ident[:])
            rhs1 = cpool.tile([P, F1 + 16], BF16)
            nc.sync.dma_start(rhs1[:], d_RHS1[:, :])
            rhs2 = cpool.tile([P, 2, F2 + 16], BF16)
            nc.sync.dma_start(rhs2[:], d_RHS2[:, :, :])
            b1row = cpool.tile([P, F1], F32)
            nc.sync.dma_start(b1row[:], d_B1[:, :])
            b2mean = cpool.tile([P, OUTW], F32)
            nc.sync.dma_start(b2mean[:], d_B2M[:, :])

            # ------- local layer-1 table (rows [feat|el|er]) + allgather ---
            with tc.tile_pool(name="t1ps", bufs=2, space="PSUM") as t1ps:
                for g in range(NG2):
                    xl = wpool.tile([P, TL2 * P], BF16, tag="xl")
                    nc.sync.dma_start(
                        xl[:], d_xTl[:, g * TL2 * P:(g + 1) * TL2 * P])
                    stg = spool.tile([P, TL2, F1 + 16], BF16, tag="stg1")
                    for q0, qn in ((0, 2), (2, 2), (4, 2), (6, 1)):
                        # 512-f32 slice pitch keeps each matmul output inside
                        # a single 2KB PSUM bank (outputs must not cross one)
                        ps = t1ps.tile([P, 2, 512], F32, tag="t1")
                        for m in range(qn):
                            nc.tensor.matmul(
                                ps[:, m, 0:F1 + 16],
                                lhsT=xl[:, (q0 + m) * P:(q0 + m + 1) * P],
                                rhs=rhs1[:], start=True, stop=True)
                        (nc.vector.tensor_copy if q0 % 4 == 0
                         else nc.scalar.copy)(
                            stg[:, q0:q0 + qn, :],
                            ps[:, 0:qn, 0:F1 + 16])
                    nc.sync.dma_start(
                        d_t1l[g * TL2:(g + 1) * TL2, :, 0:F1 + 16]
                        .rearrange("t p c -> p t c"), stg[:])

            if C > 1 and not skip_cc:
                nc.gpsimd.collective_compute(
                    "AllGather", OP.bypass,
                    replica_groups=[list(range(C))],
                    ins=[d_t1l[:, :, :]],
                    outs=[d_tab1[:, :, :, :]],
                )

            # ---------------- edge phase (shared) ----------------
            def edge_phase(layer, pspool, tps, tpp):
                F = F1 if layer == 1 else F2
                ROW = ROW1E if layer == 1 else ROW2
                tab = tab1_flat if layer == 1 else tab2_flat
                d_meta, d_midx = d_meta1, d_midx1
                PIECE = 768
                single_packet = "mp" not in os.environ.get("GAT_OPT", "mp")
                for b in range(NB):
                    midx = ipool.tile([P, MI], I8, tag="midx")
                    nc.sync.dma_start(midx[:], d_midx[:, b * MI:(b + 1) * MI])
                    meta = wpool.tile([P, MC], I8, tag="meta")
                    nc.sync.dma_start(meta[:], d_meta[:, b * MC:(b + 1) * MC])
                    il = midx[:, 0:2 * CL].bitcast(I16)
                    ih = midx[:, 2 * CL:MI].bitcast(I16)
                    erblk = ipool.tile([P, 8], BF16, tag="erblk")
                    if layer == 1:
                        nc.sync.dma_start(erblk[:],
                                          d_t1l[b, :, F1 + 8:F1 + 16])
                    else:
                        nc.sync.dma_start(erblk[:],
                                          d_t2l[b, :, F2 + 8:F2 + 16])
                    G = gpool.tile([P, NCt, ROW], BF16, tag=f"G{layer}")
                    Gfeat = G[:, :, 0:F]
                    Gel = G[:, :, F:F + 8]
                    if single_packet:
                        pieces = [(s0, min(PIECE, SLO - s0), 0)
                                  for s0 in range(0, SLO, PIECE)]
                        pieces += [(s0, min(PIECE, SHI - s0), 1)
                                   for s0 in range(0, SHI, PIECE)]
                    else:
                        pieces = [(0, SLO, 0), (0, SHI, 1)]
                    for s0, n, is_hi in pieces:
                        idxs = ih if is_hi else il
                        base = SLO if is_hi else 0
                        src_ap = tab[cfg.HALF:, :] if is_hi else tab[:, :]
                        nc.gpsimd.dma_gather(
                            out_ap=G[:, (base + s0) // P:(base + s0 + n) // P, :],
                            in_ap=src_ap,
                            idxs_ap=idxs[:, s0 // 16:(s0 + n) // 16],
                            num_idxs=n, num_idxs_reg=n, elem_size=ROW,
                            single_packet=single_packet)

                    # derive the gather one-hot ohT = oh^T on PE via a
                    # plain matmul against the identity (halves the meta DMA)
                    ohT = opool.tile([P, NCt, P], BF16, tag="ohT")
                    for i, g0 in enumerate(range(0, NCt, 4)):
                        gn = min(4, NCt - g0)
                        tp_ = tpp.tile([P, 4, P], F32, tag="ohTp")
                        for c in range(gn):
                            nc.tensor.matmul(
                                tp_[:, c, :],
                                lhsT=meta[:, (g0 + c) * P:(g0 + c + 1) * P]
                                .bitcast(FP8),
                                rhs=ident[:], start=True, stop=True)
                        (nc.vector.tensor_copy if i % 2 == 0
                         else nc.scalar.copy)(
                            ohT[:, g0:g0 + gn, :], tp_[:, 0:gn, :])
                    # er broadcast to slots
                    erps = tps.tile([P, NCt, 8], F32, tag="erps")
                    for c in range(NCt):
                        nc.tensor.matmul(
                            erps[:, c, :],
                            lhsT=ohT[:, c, :],
                            rhs=erblk[:], start=True, stop=True)
                    e_t = wpool.tile([P, NCt, 8], BF16, tag="e_t")
                    nc.vector.tensor_tensor(out=e_t[:], in0=Gel,
                                            in1=erps[:], op=OP.add)
                    # leaky_relu on DVE: es = max(e, 0.2*e)  (the Lrelu ACT
                    # table lives in a different func set than Exp; switching
                    # would reload the 1283ns act table every block)
                    ea = wpool.tile([P, NCt, 8], BF16, tag="ea")
                    nc.vector.tensor_scalar(ea[:], e_t[:], cfg.NEG, None,
                                            op0=OP.mult)
                    es = wpool.tile([P, NCt, 8], BF16, tag="es")
                    nc.vector.tensor_tensor(out=es[:], in0=e_t[:], in1=ea[:],
                                            op=OP.max)
                    MSG = gpool.tile([P, NCt, F + 8], BF16, tag=f"MSG{layer}")
                    nc.scalar.activation(MSG[:, :, F:F + 8], es[:], AF.Exp)
                    nc.vector.tensor_tensor(
                        out=MSG[:, :, 0:F].rearrange(
                            "p c (d h) -> p c d h", h=8),
                        in0=Gfeat.rearrange("p c (d h) -> p c d h", h=8),
                        in1=MSG[:, :, F:F + 8].rearrange(
                            "p c (one h) -> p c one h", one=1)
                        .to_broadcast([P, NCt, F // 8, 8]),
                        op=OP.mult)
                    ps = pspool.tile([P, F + 8], F32, tag="eps")
                    for c in range(NCt):
                        nc.tensor.matmul(
                            ps[:],
                            lhsT=meta[:, c * P:(c + 1) * P].bitcast(FP8),
                            rhs=MSG[:, c, :],
                            start=(c == 0), stop=(c == NCt - 1))
                    esum = wpool.tile([P, 8], F32, tag="esum")
                    nc.vector.tensor_scalar(esum[:], ps[:, F:F + 8], 1e-30,
                                            None, op0=OP.max)
                    inv = wpool.tile([P, 8], F32, tag="inv")
                    nc.vector.reciprocal(inv[:], esum[:])
                    yield b, ps, inv

            # ---------------- layer-1 edges + hT + local tab2 ----------------
            with tc.tile_pool(name="e1ps", bufs=2, space="PSUM") as e1ps, \
                 tc.tile_pool(name="tps", bufs=2, space="PSUM") as tps, \
                 tc.tile_pool(name="tp1", bufs=2, space="PSUM") as tpp1:
                for b, ps, inv in edge_phase(1, e1ps, tps, tpp1):
                    z = wpool.tile([P, F1], F32, tag="z")
                    nc.vector.tensor_tensor(
                        out=z[:].rearrange("p (d h) -> p d h", h=8),
                        in0=ps[:, 0:F1].rearrange("p (d h) -> p d h", h=8),
                        in1=inv[:].rearrange("p (one h) -> p one h", one=1)
                        .to_broadcast([P, F1 // 8, 8]),
                        op=OP.mult)
                    nc.vector.tensor_add(z[:], z[:], b1row[:])
                    # elu(z) = relu(z) + exp(min(z,0)) - 1
                    # min(z,0) = -relu(-z); both relu and exp run on ACT
                    zmn = wpool.tile([P, F1], BF16, tag="zmn")
                    nc.scalar.activation(zmn[:], z[:], AF.Relu, scale=-1.0)
                    zp = wpool.tile([P, F1], BF16, tag="zp")
                    nc.scalar.activation(zp[:], z[:], AF.Relu)
                    q_ = wpool.tile([P, F1], BF16, tag="q_")
                    nc.scalar.activation(q_[:], zmn[:], AF.Exp, scale=-1.0)
                    hb = wpool.tile([P, F1], BF16, tag="hb")
                    nc.vector.affine_then_add(hb[:], q_[:], zp[:], 1.0, -1.0)
                    pst = tps.tile([P, 2, P], BF16, tag="pst")
                    for q in range(2):
                        nc.tensor.transpose(pst[:, q, :],
                                            hb[:, q * P:(q + 1) * P], ident[:])
                    htp = wpool.tile([P, 2, P], BF16, tag="htp")
                    nc.vector.tensor_copy(htp[:], pst[:])
                    nc.sync.dma_start(d_hT[:, b, :, :], htp[:])

            # ---------------- local layer-2 table + allgather ----------------
            with tc.tile_pool(name="t2ps", bufs=2, space="PSUM") as t2ps:
                for g in range(NG2):
                    ht = wpool.tile([P, TL2, 2, P], BF16, tag="ht")
                    nc.sync.dma_start(ht[:], d_hT[:, g * TL2:(g + 1) * TL2,
                                                  :, :])
                    # 256-f32 slice pitch: every 144-wide output stays inside
                    # one 2KB PSUM bank
                    ps2 = t2ps.tile([P, TL2, 256], F32, tag="t2")
                    for m in range(TL2):
                        nc.tensor.matmul(ps2[:, m, 0:F2 + 16],
                                         lhsT=ht[:, m, 0, :],
                                         rhs=rhs2[:, 0, :],
                                         start=True, stop=False)
                        nc.tensor.matmul(ps2[:, m, 0:F2 + 16],
                                         lhsT=ht[:, m, 1, :],
                                         rhs=rhs2[:, 1, :],
                                         start=False, stop=True)
                    st2 = spool.tile([P, TL2, F2 + 16], BF16, tag="stg2")
                    (nc.vector.tensor_copy if g % 2 == 0 else nc.scalar.copy)(
                        st2[:], ps2[:, :, 0:F2 + 16])
                    nc.sync.dma_start(
                        d_t2l[g * TL2:(g + 1) * TL2, :, 0:F2 + 16]
                        .rearrange("t p c -> p t c"), st2[:])

            if C > 1 and not skip_cc:
                nc.gpsimd.collective_compute(
                    "AllGather", OP.bypass,
                    replica_groups=[list(range(C))],
                    ins=[d_t2l[:, :, :]],
                    outs=[d_tab2[:, :, :, :]],
                )

            # ---------------- layer-2 edges + output ----------------
            with tc.tile_pool(name="e2ps", bufs=2, space="PSUM") as e2ps, \
                 tc.tile_pool(name="tps2", bufs=2, space="PSUM") as tps2, \
                 tc.tile_pool(name="tp2", bufs=2, space="PSUM") as tpp2:
                ostage = None
                for b, ps, inv in edge_phase(2, e2ps, tps2, tpp2):
                    if b % 8 == 0:
                        ostage = spool.tile([P, 8, OUTW], F32, tag="ostage")
                    inv8 = wpool.tile([P, 8], F32, tag="inv8")
                    nc.vector.tensor_scalar(inv8[:], inv[:], 0.125, None,
                                            op0=OP.mult)
                    w_ = wpool.tile([P, OUTW, 8], F32, tag="w_")
                    nc.vector.tensor_tensor(
                        out=w_[:],
                        in0=ps[:, 0:F2].rearrange("p (d h) -> p d h", h=8),
                        in1=inv8[:].rearrange("p (one h) -> p one h", one=1)
                        .to_broadcast([P, OUTW, 8]),
                        op=OP.mult)
                    s1 = wpool.tile([P, OUTW, 4], F32, tag="s1")
                    nc.vector.tensor_add(s1[:], w_[:, :, 0:4], w_[:, :, 4:8])
                    s2 = wpool.tile([P, OUTW, 2], F32, tag="s2")
                    nc.vector.tensor_add(s2[:], s1[:, :, 0:2], s1[:, :, 2:4])
                    s3 = wpool.tile([P, OUTW], F32, tag="s3")
                    nc.vector.tensor_add(s3[:], s2[:, :, 0], s2[:, :, 1])
                    nc.vector.tensor_add(ostage[:, b % 8, :], s3[:],
                                         b2mean[:])
                    if b % 8 == 7 or b == NB - 1:
                        b0 = (b // 8) * 8
                        nt = b - b0 + 1
                        nc.sync.dma_start(
                            d_out[b0:b0 + nt, :, :]
                            .rearrange("t p c -> p t c"),
                            ostage[:, 0:nt, :])

            if dbg:
                for nm, src_t in [("dbg_t1l", d_t1l), ("dbg_tab1", d_tab1),
                                  ("dbg_hT", d_hT), ("dbg_t2l", d_t2l),
                                  ("dbg_tab2", d_tab2)]:
                    dd = nc.dram_tensor(nm, list(src_t.shape), BF16,
                                        kind="ExternalOutput")
                    sl = tuple(slice(None) for _ in src_t.shape)
                    nc.sync.dma_start(dd[sl], src_t[sl])

    nc.compile()
    return nc


# ----------------------------------------------------------------------------
_CACHE = {}


def get_built(src, dst, C=8, cfg=None):
    key = (hash(src.tobytes()), hash(dst.tobytes()), C)
    if key not in _CACHE:
        if cfg is None:
            cfg = GATCfg(C=C)
        idx = prep_indices(src, dst, cfg)
        nc = build_module(cfg)
        _CACHE[key] = (cfg, idx, nc)
    return _CACHE[key]


_EXECC = {}


def _get_exec(key, nc, n_cores):
    """Persistent jit(shard_map(bass_exec)) so repeated kernel() calls skip
    retracing/recompiling."""
    if key in _EXECC:
        return _EXECC[key]
    import jax
    from jax.experimental.shard_map import shard_map
    from jax.sharding import Mesh, NamedSharding, PartitionSpec
    from concourse import bass2jax
    bass2jax.install_neuronx_cc_hook()
    partition_name = (nc.partition_id_tensor.name
                      if nc.partition_id_tensor else None)
    in_names, out_names, out_avals, zero_shapes = [], [], [], []
    for alloc in nc.m.functions[0].allocations:
        if not isinstance(alloc, mybir.MemoryLocationSet):
            continue
        name = alloc.memorylocations[0].name
        if alloc.kind == "ExternalInput":
            if name != partition_name:
                in_names.append(name)
        elif alloc.kind == "ExternalOutput":
            out_names.append(name)
            shape = tuple(alloc.tensor_shape)
            dtype = mybir.dt.np(alloc.dtype)
            out_avals.append(jax.core.ShapedArray(shape, dtype))
            zero_shapes.append((shape, dtype))
    n_params = len(in_names)
    in_names_all = list(in_names) + out_names + (
        [partition_name] if partition_name else [])

    def _body(*args):
        ops = list(args)
        if partition_name:
            ops.append(bass2jax.partition_id_tensor())
        outs = bass2jax._bass_exec_p.bind(
            *ops, out_avals=tuple(out_avals), in_names=tuple(in_names_all),
            out_names=tuple(out_names), lowering_input_output_aliases=(),
            sim_require_finite=True, sim_require_nnan=True, nc=nc)
        return tuple(outs)

    devices = jax.devices()[:n_cores]
    mesh = Mesh(np.asarray(devices), ("core",))
    nout = len(out_names)
    f = jax.jit(shard_map(
        _body, mesh=mesh,
        in_specs=(PartitionSpec("core"),) * (n_params + nout),
        out_specs=(PartitionSpec("core"),) * nout, check_rep=False),
        keep_unused=True)
    sh = NamedSharding(mesh, PartitionSpec("core"))
    ent = dict(f=f, in_names=in_names, out_names=out_names,
               zero_shapes=zero_shapes, sh=sh, argcache=None)
    _EXECC[key] = ent
    return ent


def kernel(**inputs) -> np.ndarray:
    import jax
    src = np.asarray(inputs["src"], np.int32)
    dst = np.asarray(inputs["dst"], np.int32)
    x = np.asarray(inputs["x"])
    base = GATCfg(N=int(x.shape[0]), C=8, IN=int(x.shape[1]))
    cfg, idx, nc = get_built(src, dst, C=8, cfg=base)
    in_maps = host_inputs(inputs, cfg, idx)
    key = (hash(src.tobytes()), hash(dst.tobytes()), cfg.C)
    ent = _get_exec(key, nc, cfg.C)
    C = cfg.C
    concat_in = [np.ascontiguousarray(
        np.concatenate([in_maps[c][nm] for c in range(C)], axis=0))
        for nm in ent["in_names"]]
    hashes = tuple(hash(a.tobytes()) for a in concat_in)
    if ent["argcache"] is None or ent["argcache"][0] != hashes:
        zeros = [np.zeros((C * sh0[0], *sh0[1:]), dt)
                 for sh0, dt in ent["zero_shapes"]]
        args = [jax.device_put(a, ent["sh"]) for a in concat_in + zeros]
        ent["argcache"] = (hashes, args)
    args = ent["argcache"][1]
    outs = ent["f"](*args)
    jax.block_until_ready(outs)
    oi = ent["out_names"].index("out")
    out = np.asarray(outs[oi]).reshape(C, cfg.Nlp, cfg.OUT)
    full = out[:, :cfg.Nloc, :].reshape(-1, cfg.OUT)[:cfg.N]
    return np.ascontiguousarray(full.astype(np.float32))


# revision 60
# speedup vs baseline: 1.0265x; 1.0216x over previous
"""2-layer GAT (graph attention) Bass/Tile kernel for Trainium2, 8-core SPMD.

Sharding: nodes partitioned contiguously across cores; edges assigned to the
core owning their dst, sorted by dst, grouped into 128-dst blocks with
uniformly padded lo/hi slot halves (int16 gather index limit) so all cores
share one SPMD module.

Per core: both layers' node-feature tables are built LOCALLY (each core
transforms only its own nodes: x@[W1|A1] resp. h@[W2|A2], PSUM slices on a
512-f32 pitch so no matmul output crosses a 2KB PSUM bank) and AllGather'ed
into rank-major gather tables — the halo exchange of transformed src node
features.  tab1 rows [feat(d-major) | el | er] bf16 (768B rows for
dma_gather), tab2 rows 512B.  Because both tables share the rank-major id
space, one meta/midx tensor pair serves both edge phases.  Edge phase per
block: one int8 meta DMA carries the pre-built fp8 scatter one-hot (PE
accepts fp8 lhsT with bf16 rhs, so no per-chunk DVE is_equal builds); the
gather one-hot is derived on-chip as oh^T via plain PE matmuls against the
identity (halving the meta traffic), with the PSUM round-trip copies
alternating between DVE and ACT; a small separate DMA carries the int16
gather indices; multi-packet dma_gather fetches src rows into one fused
lo+hi tile; PE matmuls broadcast er to slots and scatter-add messages +
exp-sums into PSUM.  Features use a d-major (d,h) layout so the
attention-weight broadcast is along a non-innermost axis, keeping the DVE
2x mode for the message multiply; leaky-relu runs on DVE (the Lrelu ACT
table lives in a different func set than Exp and would reload every
block), exp/relu on ACT.  Per-dst softmax normalization happens after the
reduction (max-subtraction skipped; |e| is O(1)).

Host precomputes (numpy, index/layout-only plus weight repacking): slot
maps, fp8 one-hots, wrapped int16 gather indices, d-major-permuted
[W | W@al | W@ar] rhs blocks, bias rows, and the bf16 x transpose.
"""

import os

import numpy as np

import concourse.bacc as bacc
import concourse.bass as bass
import concourse.mybir as mybir
import concourse.tile as tile
from concourse.masks import make_identity

F32 = mybir.dt.float32
BF16 = mybir.dt.bfloat16
I32 = mybir.dt.int32
I16 = mybir.dt.int16
I8 = mybir.dt.int8
FP8 = mybir.dt.float8e4
AF = mybir.ActivationFunctionType
OP = mybir.AluOpType

P = 128
HALF_LIMIT = 32768  # int16 gather index limit
ONE_FP8 = 0x38      # float8e4m3 encoding of 1.0


class GATCfg:
    def __init__(self, N=50000, C=8, IN=128, HID=32, HEADS=8, OUT=16, NEG=0.2):
        self.N, self.C, self.IN = N, C, IN
        self.HID, self.HEADS, self.OUT, self.NEG = HID, HEADS, OUT, NEG
        self.F1 = HEADS * HID        # 256
        self.F2 = HEADS * OUT        # 128
        self.FP8T1 = os.environ.get("GAT_FP8T1", "0") == "1"
        # tab1 row in BYTES: [feat | el]; fp8 feat: 256+16 -> 512B rows,
        # bf16 feat: 512+16 -> 768B rows (dma_gather needs 256B multiples)
        self.ROW1B = 512 if self.FP8T1 else 768
        self.T1USED = (256 if self.FP8T1 else 512) + 16
        self.ROW2 = 256              # bf16 elems; 512B rows (used: 128+8+8)
        self.Nloc = (N + C - 1) // C
        self.NB = (self.Nloc + P - 1) // P
        self.Nlp = self.NB * P
        self.NP1 = ((N + 511) // 512) * 512
        self.NP2 = C * self.Nlp
        NPmax = max(self.NP1, self.NP2)
        h = (NPmax // 2 + P - 1) // P * P
        self.HALF = (min(HALF_LIMIT, max(h, NPmax - HALF_LIMIT))
                     if NPmax > HALF_LIMIT else NPmax)
        self.HALF = max(self.HALF, NPmax - HALF_LIMIT)
        self.SLO = 0
        self.SHI = 0

    @property
    def NCt(self):
        return (self.SLO + self.SHI) // P

    def meta_cols(self):
        return self.NCt * P

    def midx_cols(self):
        return 2 * (self.SLO // 16) + 2 * (self.SHI // 16)


def _wrap16(vals_slots, S):
    """[S] slot-ordered ints -> [128, S//16] 16-wrapped, replicated 8x."""
    a = vals_slots.reshape(S // 16, 16)
    out = np.zeros((128, S // 16), np.int16)
    blkcols = a.T.astype(np.int16)  # [16, S//16]
    for r in range(8):
        out[r * 16:(r + 1) * 16, :] = blkcols
    return out


def prep_indices(src, dst, cfg):
    """Host index-only preprocessing: slot maps, gather indices, fp8 one-hots."""
    C, Nloc, NB = cfg.C, cfg.Nloc, cfg.NB
    src = np.asarray(src).astype(np.int64)
    dst = np.asarray(dst).astype(np.int64)
    core = dst // Nloc
    dloc = dst - core * Nloc
    blk = dloc // P
    dblk = dloc - blk * P
    key = core * NB + blk
    order = np.argsort(key, kind="stable")
    counts = np.bincount(key, minlength=C * NB)
    starts = np.zeros(C * NB + 1, np.int64)
    np.cumsum(counts, out=starts[1:])

    r_of = src // Nloc
    srcp = r_of * cfg.Nlp + (src - r_of * Nloc)   # layer-2 rank-major id

    HALFc = cfg.HALF

    def lohi_max(ids):
        lo_max = hi_max = 0
        for k in range(C * NB):
            e = order[starts[k]:starts[k + 1]]
            n_lo = int((ids[e] < HALFc).sum())
            lo_max = max(lo_max, n_lo)
            hi_max = max(hi_max, e.size - n_lo)
        return lo_max, hi_max

    lo1, hi1 = lohi_max(srcp)
    SLO = ((max(lo1, 1) + P - 1) // P) * P
    SHI = ((max(hi1, 1) + P - 1) // P) * P
    cfg.SLO, cfg.SHI = SLO, SHI
    NCt = cfg.NCt
    CL, CH = SLO // 16, SHI // 16
    MC = cfg.meta_cols()

    out = {}
    MI = 2 * CL + 2 * CH
    for layer, ids in ((1, srcp),):
        meta = np.zeros((C, NB, 128, MC), np.uint8)
        midx = np.zeros((C, NB, 128, MI), np.uint8)
        for c in range(C):
            for b in range(NB):
                k = c * NB + b
                e = order[starts[k]:starts[k + 1]]
                v = ids[e]
                m = v < HALFc
                elo, ehi = e[m], e[~m]
                ilo = np.zeros(SLO, np.int64)
                ihi = np.zeros(SHI, np.int64)
                ilo[:elo.size] = v[m]
                ihi[:ehi.size] = v[~m] - HALFc
                # slot ids: lo at [0,nlo), hi at [SLO, SLO+nhi)
                sl = np.concatenate([np.arange(elo.size),
                                     SLO + np.arange(ehi.size)])
                dv = dblk[np.concatenate([elo, ehi])]
                oh = np.zeros((128, NCt * P), np.uint8)
                # oh[p, c*128 + m] = (m == d(slot c*128+p))
                oh[sl % P, (sl // P) * P + dv] = ONE_FP8
                wl = _wrap16(ilo, SLO).view(np.uint8).reshape(128, 2 * CL)
                wh = _wrap16(ihi, SHI).view(np.uint8).reshape(128, 2 * CH)
                meta[c, b] = oh
                midx[c, b] = np.concatenate([wl, wh], axis=1)
        out[f"meta{layer}"] = np.ascontiguousarray(
            meta.transpose(0, 2, 1, 3).reshape(C, 128, NB * MC))
        out[f"midx{layer}"] = np.ascontiguousarray(
            midx.transpose(0, 2, 1, 3).reshape(C, 128, NB * MI))
    return out


def _dperm(H, D):
    """permutation p with p[d*H+h] = h*D+d (d-major ordering)."""
    idx = np.arange(H * D).reshape(H, D).T.reshape(-1)
    return idx


def host_inputs(inputs, cfg, idx):
    import ml_dtypes
    BF = ml_dtypes.bfloat16
    H, D1, D2 = cfg.HEADS, cfg.HID, cfg.OUT
    F1, F2 = cfg.F1, cfg.F2
    p1 = _dperm(H, D1)   # F1 perm
    p2 = _dperm(H, D2)   # F2 perm

    x = np.asarray(inputs["x"], np.float32)
    xT = np.zeros((cfg.IN, cfg.N), np.float32)
    xT[:, :cfg.N] = np.ascontiguousarray(x.T)

    W1 = np.asarray(inputs["W1"], np.float32)
    W2 = np.asarray(inputs["W2"], np.float32)
    al1 = np.asarray(inputs["al1"], np.float32)
    ar1 = np.asarray(inputs["ar1"], np.float32)
    al2 = np.asarray(inputs["al2"], np.float32)
    ar2 = np.asarray(inputs["ar2"], np.float32)
    b1 = np.asarray(inputs["b1"], np.float32)
    b2 = np.asarray(inputs["b2"], np.float32)

    # el/er projection matrices [IN, 16]: col h = W1[:, head h] @ al1[h]
    A1 = np.zeros((cfg.IN, 16), np.float32)
    for h in range(H):
        A1[:, h] = W1[:, h * D1:(h + 1) * D1] @ al1[h]
        A1[:, 8 + h] = W1[:, h * D1:(h + 1) * D1] @ ar1[h]
    RHS1 = np.concatenate([W1[:, p1], A1], axis=1)          # [IN, 272]

    # layer2: rows of W2 permuted to d-major F1; cols to d-major F2
    W2p = W2[p1][:, p2]                                      # [256, 128]
    A2 = np.zeros((F1, 16), np.float32)
    for h in range(H):
        A2[:, h] = W2[:, h * D2:(h + 1) * D2] @ al2[h]
        A2[:, 8 + h] = W2[:, h * D2:(h + 1) * D2] @ ar2[h]
    A2 = A2[p1]
    RHS2f = np.concatenate([W2p, A2], axis=1)                # [256, 144]
    # pack as [128, 2, 144] (partition = K within half)
    RHS2 = np.ascontiguousarray(
        RHS2f.reshape(2, 128, 144).transpose(1, 0, 2))

    B1 = np.broadcast_to(b1[p1][None, :], (128, F1)).astype(np.float32)
    b2m = b2.reshape(H, D2).mean(axis=0)                     # [16]
    B2M = np.broadcast_to(b2m[None, :], (128, D2)).astype(np.float32)

    in_maps = []
    for c in range(cfg.C):
        xTl = np.zeros((cfg.IN, cfg.Nlp), np.float32)
        lo = c * cfg.Nloc
        hi = min(cfg.N, lo + cfg.Nloc)
        xTl[:, :hi - lo] = xT[:, lo:hi]
        in_maps.append({
            "xTl": xTl.astype(BF),
            "RHS1": RHS1.astype(BF), "RHS2": RHS2.astype(BF),
            "B1": np.ascontiguousarray(B1),
            "B2M": np.ascontiguousarray(B2M),
            "meta1": idx["meta1"][c].view(np.int8),
            "midx1": idx["midx1"][c].view(np.int8),
        })
    return in_maps


def build_module(cfg, dbg=False, skip_cc=False):
    nc = bacc.Bacc("TRN2", target_bir_lowering=False, debug=False,
                   num_devices=cfg.C)
    C, NB, Nlp = cfg.C, cfg.NB, cfg.Nlp
    F1, F2, ROW2 = cfg.F1, cfg.F2, cfg.ROW2
    ROW1E = cfg.ROW1B // 2       # tab1 row in bf16 elems (bf16 mode only)
    assert not cfg.FP8T1
    SLO, SHI, NCt = cfg.SLO, cfg.SHI, cfg.NCt
    NLOC, NHIC = SLO // P, SHI // P
    CL, CH = SLO // 16, SHI // 16
    MC = cfg.meta_cols()
    MI = cfg.midx_cols()
    OUTW = cfg.OUT
    NT1 = cfg.NP1 // P            # 392 layer-1 table tiles
    TB1 = 8                       # tiles per L1 table DMA group
    NG1 = NT1 // TB1              # 49
    TL2 = 7                       # tiles per L2 table load
    NG2 = NB // TL2               # 7

    d_xTl = nc.dram_tensor("xTl", [cfg.IN, Nlp], BF16, kind="ExternalInput")
    d_RHS1 = nc.dram_tensor("RHS1", [cfg.IN, F1 + 16], BF16,
                            kind="ExternalInput")
    d_RHS2 = nc.dram_tensor("RHS2", [P, 2, F2 + 16], BF16,
                            kind="ExternalInput")
    d_B1 = nc.dram_tensor("B1", [P, F1], F32, kind="ExternalInput")
    d_B2M = nc.dram_tensor("B2M", [P, OUTW], F32, kind="ExternalInput")
    d_meta1 = nc.dram_tensor("meta1", [P, NB * MC], I8, kind="ExternalInput")
    d_midx1 = nc.dram_tensor("midx1", [P, NB * MI], I8, kind="ExternalInput")
    d_out = nc.dram_tensor("out", [NB, P, OUTW], F32, kind="ExternalOutput")

    d_t1l = nc.dram_tensor("t1l", [NB, P, ROW1E], BF16, kind="Internal")
    d_tab1 = nc.dram_tensor("tab1", [C, NB, P, ROW1E], BF16, kind="Internal",
                            addr_space="Shared" if C > 4 else "Local")
    d_t2l = nc.dram_tensor("t2l", [NB, P, ROW2], BF16, kind="Internal")
    d_tab2 = nc.dram_tensor("tab2", [C, NB, P, ROW2], BF16, kind="Internal",
                            addr_space="Shared" if C > 4 else "Local")
    d_hT = nc.dram_tensor("hT", [P, NB, 2, P], BF16, kind="Internal")

    tab1_flat = d_tab1.rearrange("r t p c -> (r t p) c")
    tab2_flat = d_tab2.rearrange("r t p c -> (r t p) c")

    with tile.TileContext(nc) as tc:
        with (
            tc.tile_pool(name="const", bufs=1) as cpool,
            tc.tile_pool(name="work", bufs=6) as wpool,
            tc.tile_pool(name="gath", bufs=2) as gpool,
            tc.tile_pool(name="stage", bufs=4) as spool,
            tc.tile_pool(name="idx", bufs=8) as ipool,
            tc.tile_pool(name="oht", bufs=4) as opool,
        ):
            # ---------------- constants ----------------
            ident = cpool.tile([P, P], BF16)
            make_identity(nc, ident[:])
            rhs1 = cpool.tile([P, F1 + 16], BF16)
            nc.sync.dma_start(rhs1[:], d_RHS1[:, :])
            rhs2 = cpool.tile([P, 2, F2 + 16], BF16)
            nc.sync.dma_start(rhs2[:], d_RHS2[:, :, :])
            b1row = cpool.tile([P, F1], F32)
            nc.sync.dma_start(b1row[:], d_B1[:, :])
            b2mean = cpool.tile([P, OUTW], F32)
            nc.sync.dma_start(b2mean[:], d_B2M[:, :])

            # ------- local layer-1 table (rows [feat|el|er]) + allgather ---
            with tc.tile_pool(name="t1ps", bufs=2, space="PSUM") as t1ps:
                for g in range(NG2):
                    xl = wpool.tile([P, TL2 * P], BF16, tag="xl")
                    nc.sync.dma_start(
                        xl[:], d_xTl[:, g * TL2 * P:(g + 1) * TL2 * P])
                    stg = spool.tile([P, TL2, F1 + 16], BF16, tag="stg1")
                    for q0, qn in ((0, 2), (2, 2), (4, 2), (6, 1)):
                        # 512-f32 slice pitch keeps each matmul output inside
                        # a single 2KB PSUM bank (outputs must not cross one)
                        ps = t1ps.tile([P, 2, 512], F32, tag="t1")
                        for m in range(qn):
                            nc.tensor.matmul(
                                ps[:, m, 0:F1 + 16],
                                lhsT=xl[:, (q0 + m) * P:(q0 + m + 1) * P],
                                rhs=rhs1[:], start=True, stop=True)
                        (nc.vector.tensor_copy if q0 % 4 == 0
                         else nc.scalar.copy)(
                            stg[:, q0:q0 + qn, :],
                            ps[:, 0:qn, 0:F1 + 16])
                    nc.sync.dma_start(
                        d_t1l[g * TL2:(g + 1) * TL2, :, 0:F1 + 16]
                        .rearrange("t p c -> p t c"), stg[:])

            if C > 1 and not skip_cc:
                nc.gpsimd.collective_compute(
                    "AllGather", OP.bypass,
                    replica_groups=[list(range(C))],
                    ins=[d_t1l[:, :, :]],
                    outs=[d_tab1[:, :, :, :]],
                )

            # ---------------- edge phase (shared) ----------------
            def edge_phase(layer, pspool, tps, tpp):
                F = F1 if layer == 1 else F2
                ROW = ROW1E if layer == 1 else ROW2
                tab = tab1_flat if layer == 1 else tab2_flat
                d_meta, d_midx = d_meta1, d_midx1
                PIECE = 768
                single_packet = "mp" not in os.environ.get("GAT_OPT", "mp")
                for b in range(NB):
                    midx = ipool.tile([P, MI], I8, tag="midx")
                    nc.sync.dma_start(midx[:], d_midx[:, b * MI:(b + 1) * MI])
                    meta = wpool.tile([P, MC], I8, tag="meta")
                    nc.sync.dma_start(meta[:], d_meta[:, b * MC:(b + 1) * MC])
                    il = midx[:, 0:2 * CL].bitcast(I16)
                    ih = midx[:, 2 * CL:MI].bitcast(I16)
                    erblk = ipool.tile([P, 8], BF16, tag="erblk")
                    if layer == 1:
                        nc.sync.dma_start(erblk[:],
                                          d_t1l[b, :, F1 + 8:F1 + 16])
                    else:
                        nc.sync.dma_start(erblk[:],
                                          d_t2l[b, :, F2 + 8:F2 + 16])
                    G = gpool.tile([P, NCt, ROW], BF16, tag=f"G{layer}")
                    Gfeat = G[:, :, 0:F]
                    Gel = G[:, :, F:F + 8]
                    if single_packet:
                        pieces = [(s0, min(PIECE, SLO - s0), 0)
                                  for s0 in range(0, SLO, PIECE)]
                        pieces += [(s0, min(PIECE, SHI - s0), 1)
                                   for s0 in range(0, SHI, PIECE)]
                    else:
                        pieces = [(0, SLO, 0), (0, SHI, 1)]
                    for s0, n, is_hi in pieces:
                        idxs = ih if is_hi else il
                        base = SLO if is_hi else 0
                        src_ap = tab[cfg.HALF:, :] if is_hi else tab[:, :]
                        nc.gpsimd.dma_gather(
                            out_ap=G[:, (base + s0) // P:(base + s0 + n) // P, :],
                            in_ap=src_ap,
                            idxs_ap=idxs[:, s0 // 16:(s0 + n) // 16],
                            num_idxs=n, num_idxs_reg=n, elem_size=ROW,
                            single_packet=single_packet)

                    # derive the gather one-hot ohT = oh^T on PE via a
                    # plain matmul against the identity (halves the meta DMA)
                    ohT = opool.tile([P, NCt, P], BF16, tag="ohT")
                    for i, g0 in enumerate(range(0, NCt, 4)):
                        gn = min(4, NCt - g0)
                        tp_ = tpp.tile([P, 4, P], F32, tag="ohTp")
                        for c in range(gn):
                            nc.tensor.matmul(
                                tp_[:, c, :],
                                lhsT=meta[:, (g0 + c) * P:(g0 + c + 1) * P]
                                .bitcast(FP8),
                                rhs=ident[:], start=True, stop=True)
                        (nc.scalar.copy if i % 2 == 0
                         else nc.vector.tensor_copy)(
                            ohT[:, g0:g0 + gn, :], tp_[:, 0:gn, :])
                    # er broadcast to slots
                    erps = tps.tile([P, NCt, 8], F32, tag="erps")
                    for c in range(NCt):
                        nc.tensor.matmul(
                            erps[:, c, :],
                            lhsT=ohT[:, c, :],
                            rhs=erblk[:], start=True, stop=True)
                    e_t = wpool.tile([P, NCt, 8], BF16, tag="e_t")
                    nc.vector.tensor_tensor(out=e_t[:], in0=Gel,
                                            in1=erps[:], op=OP.add)
                    # leaky_relu on DVE: es = max(e, 0.2*e)  (the Lrelu ACT
                    # table lives in a different func set than Exp; switching
                    # would reload the 1283ns act table every block)
                    ea = wpool.tile([P, NCt, 8], BF16, tag="ea")
                    nc.vector.tensor_scalar(ea[:], e_t[:], cfg.NEG, None,
                                            op0=OP.mult)
                    es = wpool.tile([P, NCt, 8], BF16, tag="es")
                    nc.vector.tensor_tensor(out=es[:], in0=e_t[:], in1=ea[:],
                                            op=OP.max)
                    MSG = gpool.tile([P, NCt, F + 8], BF16, tag=f"MSG{layer}")
                    nc.scalar.activation(MSG[:, :, F:F + 8], es[:], AF.Exp)
                    nc.vector.tensor_tensor(
                        out=MSG[:, :, 0:F].rearrange(
                            "p c (d h) -> p c d h", h=8),
                        in0=Gfeat.rearrange("p c (d h) -> p c d h", h=8),
                        in1=MSG[:, :, F:F + 8].rearrange(
                            "p c (one h) -> p c one h", one=1)
                        .to_broadcast([P, NCt, F // 8, 8]),
                        op=OP.mult)
                    ps = pspool.tile([P, F + 8], F32, tag="eps")
                    for c in range(NCt):
                        nc.tensor.matmul(
                            ps[:],
                            lhsT=meta[:, c * P:(c + 1) * P].bitcast(FP8),
                            rhs=MSG[:, c, :],
                            start=(c == 0), stop=(c == NCt - 1))
                    esum = wpool.tile([P, 8], F32, tag="esum")
                    nc.vector.tensor_scalar(esum[:], ps[:, F:F + 8], 1e-30,
                                            None, op0=OP.max)
                    inv = wpool.tile([P, 8], F32, tag="inv")
                    nc.vector.reciprocal(inv[:], esum[:])
                    yield b, ps, inv

            # ---------------- layer-1 edges + hT + local tab2 ----------------
            with tc.tile_pool(name="e1ps", bufs=2, space="PSUM") as e1ps, \
                 tc.tile_pool(name="tps", bufs=2, space="PSUM") as tps, \
                 tc.tile_pool(name="tp1", bufs=2, space="PSUM") as tpp1:
                for b, ps, inv in edge_phase(1, e1ps, tps, tpp1):
                    z = wpool.tile([P, F1], F32, tag="z")
                    nc.vector.tensor_tensor(
                        out=z[:].rearrange("p (d h) -> p d h", h=8),
                        in0=ps[:, 0:F1].rearrange("p (d h) -> p d h", h=8),
                        in1=inv[:].rearrange("p (one h) -> p one h", one=1)
                        .to_broadcast([P, F1 // 8, 8]),
                        op=OP.mult)
                    nc.vector.tensor_add(z[:], z[:], b1row[:])
                    # elu(z) = relu(z) + exp(min(z,0)) - 1
                    # min(z,0) = -relu(-z); both relu and exp run on ACT
                    zmn = wpool.tile([P, F1], BF16, tag="zmn")
                    nc.scalar.activation(zmn[:], z[:], AF.Relu, scale=-1.0)
                    zp = wpool.tile([P, F1], BF16, tag="zp")
                    nc.scalar.activation(zp[:], z[:], AF.Relu)
                    q_ = wpool.tile([P, F1], BF16, tag="q_")
                    nc.scalar.activation(q_[:], zmn[:], AF.Exp, scale=-1.0)
                    hb = wpool.tile([P, F1], BF16, tag="hb")
                    nc.vector.affine_then_add(hb[:], q_[:], zp[:], 1.0, -1.0)
                    pst = tps.tile([P, 2, P], BF16, tag="pst")
                    for q in range(2):
                        nc.tensor.transpose(pst[:, q, :],
                                            hb[:, q * P:(q + 1) * P], ident[:])
                    htp = wpool.tile([P, 2, P], BF16, tag="htp")
                    nc.vector.tensor_copy(htp[:], pst[:])
                    nc.sync.dma_start(d_hT[:, b, :, :], htp[:])

            # ---------------- local layer-2 table + allgather ----------------
            with tc.tile_pool(name="t2ps", bufs=2, space="PSUM") as t2ps:
                for g in range(NG2):
                    ht = wpool.tile([P, TL2, 2, P], BF16, tag="ht")
                    nc.sync.dma_start(ht[:], d_hT[:, g * TL2:(g + 1) * TL2,
                                                  :, :])
                    # 256-f32 slice pitch: every 144-wide output stays inside
                    # one 2KB PSUM bank
                    ps2 = t2ps.tile([P, TL2, 256], F32, tag="t2")
                    for m in range(TL2):
                        nc.tensor.matmul(ps2[:, m, 0:F2 + 16],
                                         lhsT=ht[:, m, 0, :],
                                         rhs=rhs2[:, 0, :],
                                         start=True, stop=False)
                        nc.tensor.matmul(ps2[:, m, 0:F2 + 16],
                                         lhsT=ht[:, m, 1, :],
                                         rhs=rhs2[:, 1, :],
                                         start=False, stop=True)
                    st2 = spool.tile([P, TL2, F2 + 16], BF16, tag="stg2")
                    (nc.vector.tensor_copy if g % 2 == 0 else nc.scalar.copy)(
                        st2[:], ps2[:, :, 0:F2 + 16])
                    nc.sync.dma_start(
                        d_t2l[g * TL2:(g + 1) * TL2, :, 0:F2 + 16]
                        .rearrange("t p c -> p t c"), st2[:])

            if C > 1 and not skip_cc:
                nc.gpsimd.collective_compute(
                    "AllGather", OP.bypass,
                    replica_groups=[list(range(C))],
                    ins=[d_t2l[:, :, :]],
                    outs=[d_tab2[:, :, :, :]],
                )

            # ---------------- layer-2 edges + output ----------------
            with tc.tile_pool(name="e2ps", bufs=2, space="PSUM") as e2ps, \
                 tc.tile_pool(name="tps2", bufs=3, space="PSUM") as tps2, \
                 tc.tile_pool(name="tp2", bufs=3, space="PSUM") as tpp2:
                ostage = None
                for b, ps, inv in edge_phase(2, e2ps, tps2, tpp2):
                    if b % 8 == 0:
                        ostage = spool.tile([P, 8, OUTW], F32, tag="ostage")
                    inv8 = wpool.tile([P, 8], F32, tag="inv8")
                    nc.vector.tensor_scalar(inv8[:], inv[:], 0.125, None,
                                            op0=OP.mult)
                    w_ = wpool.tile([P, OUTW, 8], F32, tag="w_")
                    nc.vector.tensor_tensor(
                        out=w_[:],
                        in0=ps[:, 0:F2].rearrange("p (d h) -> p d h", h=8),
                        in1=inv8[:].rearrange("p (one h) -> p one h", one=1)
                        .to_broadcast([P, OUTW, 8]),
                        op=OP.mult)
                    s1 = wpool.tile([P, OUTW, 4], F32, tag="s1")
                    nc.vector.tensor_add(s1[:], w_[:, :, 0:4], w_[:, :, 4:8])
                    s2 = wpool.tile([P, OUTW, 2], F32, tag="s2")
                    nc.vector.tensor_add(s2[:], s1[:, :, 0:2], s1[:, :, 2:4])
                    s3 = wpool.tile([P, OUTW], F32, tag="s3")
                    nc.vector.tensor_add(s3[:], s2[:, :, 0], s2[:, :, 1])
                    nc.vector.tensor_add(ostage[:, b % 8, :], s3[:],
                                         b2mean[:])
                    if b % 8 == 7 or b == NB - 1:
                        b0 = (b // 8) * 8
                        nt = b - b0 + 1
                        nc.sync.dma_start(
                            d_out[b0:b0 + nt, :, :]
                            .rearrange("t p c -> p t c"),
                            ostage[:, 0:nt, :])

            if dbg:
                for nm, src_t in [("dbg_t1l", d_t1l), ("dbg_tab1", d_tab1),
                                  ("dbg_hT", d_hT), ("dbg_t2l", d_t2l),
                                  ("dbg_tab2", d_tab2)]:
                    dd = nc.dram_tensor(nm, list(src_t.shape), BF16,
                                        kind="ExternalOutput")
                    sl = tuple(slice(None) for _ in src_t.shape)
                    nc.sync.dma_start(dd[sl], src_t[sl])

    nc.compile()
    return nc


# ----------------------------------------------------------------------------
_CACHE = {}


def get_built(src, dst, C=8, cfg=None):
    key = (hash(src.tobytes()), hash(dst.tobytes()), C)
    if key not in _CACHE:
        if cfg is None:
            cfg = GATCfg(C=C)
        idx = prep_indices(src, dst, cfg)
        nc = build_module(cfg)
        _CACHE[key] = (cfg, idx, nc)
    return _CACHE[key]


_EXECC = {}


def _get_exec(key, nc, n_cores):
    """Persistent jit(shard_map(bass_exec)) so repeated kernel() calls skip
    retracing/recompiling."""
    if key in _EXECC:
        return _EXECC[key]
    import jax
    from jax.experimental.shard_map import shard_map
    from jax.sharding import Mesh, NamedSharding, PartitionSpec
    from concourse import bass2jax
    bass2jax.install_neuronx_cc_hook()
    partition_name = (nc.partition_id_tensor.name
                      if nc.partition_id_tensor else None)
    in_names, out_names, out_avals, zero_shapes = [], [], [], []
    for alloc in nc.m.functions[0].allocations:
        if not isinstance(alloc, mybir.MemoryLocationSet):
            continue
        name = alloc.memorylocations[0].name
        if alloc.kind == "ExternalInput":
            if name != partition_name:
                in_names.append(name)
        elif alloc.kind == "ExternalOutput":
            out_names.append(name)
            shape = tuple(alloc.tensor_shape)
            dtype = mybir.dt.np(alloc.dtype)
            out_avals.append(jax.core.ShapedArray(shape, dtype))
            zero_shapes.append((shape, dtype))
    n_params = len(in_names)
    in_names_all = list(in_names) + out_names + (
        [partition_name] if partition_name else [])

    def _body(*args):
        ops = list(args)
        if partition_name:
            ops.append(bass2jax.partition_id_tensor())
        outs = bass2jax._bass_exec_p.bind(
            *ops, out_avals=tuple(out_avals), in_names=tuple(in_names_all),
            out_names=tuple(out_names), lowering_input_output_aliases=(),
            sim_require_finite=True, sim_require_nnan=True, nc=nc)
        return tuple(outs)

    devices = jax.devices()[:n_cores]
    mesh = Mesh(np.asarray(devices), ("core",))
    nout = len(out_names)
    f = jax.jit(shard_map(
        _body, mesh=mesh,
        in_specs=(PartitionSpec("core"),) * (n_params + nout),
        out_specs=(PartitionSpec("core"),) * nout, check_rep=False),
        keep_unused=True)
    sh = NamedSharding(mesh, PartitionSpec("core"))
    ent = dict(f=f, in_names=in_names, out_names=out_names,
               zero_shapes=zero_shapes, sh=sh, argcache=None)
    _EXECC[key] = ent
    return ent


def kernel(**inputs) -> np.ndarray:
    import jax
    src = np.asarray(inputs["src"], np.int32)
    dst = np.asarray(inputs["dst"], np.int32)
    x = np.asarray(inputs["x"])
    base = GATCfg(N=int(x.shape[0]), C=8, IN=int(x.shape[1]))
    cfg, idx, nc = get_built(src, dst, C=8, cfg=base)
    in_maps = host_inputs(inputs, cfg, idx)
    key = (hash(src.tobytes()), hash(dst.tobytes()), cfg.C)
    ent = _get_exec(key, nc, cfg.C)
    C = cfg.C
    concat_in = [np.ascontiguousarray(
        np.concatenate([in_maps[c][nm] for c in range(C)], axis=0))
        for nm in ent["in_names"]]
    hashes = tuple(hash(a.tobytes()) for a in concat_in)
    if ent["argcache"] is None or ent["argcache"][0] != hashes:
        zeros = [np.zeros((C * sh0[0], *sh0[1:]), dt)
                 for sh0, dt in ent["zero_shapes"]]
        args = [jax.device_put(a, ent["sh"]) for a in concat_in + zeros]
        ent["argcache"] = (hashes, args)
    args = ent["argcache"][1]
    outs = ent["f"](*args)
    jax.block_until_ready(outs)
    oi = ent["out_names"].index("out")
    out = np.asarray(outs[oi]).reshape(C, cfg.Nlp, cfg.OUT)
    full = out[:, :cfg.Nloc, :].reshape(-1, cfg.OUT)[:cfg.N]
    return np.ascontiguousarray(full.astype(np.float32))


# revision 63
# speedup vs baseline: 1.0331x; 1.0064x over previous
"""2-layer GAT (graph attention) Bass/Tile kernel for Trainium2, 8-core SPMD.

Sharding: nodes partitioned contiguously across cores; edges assigned to the
core owning their dst, sorted by dst, grouped into 128-dst blocks with
uniformly padded lo/hi slot halves (int16 gather index limit) so all cores
share one SPMD module.

Per core: both layers' node-feature tables are built LOCALLY (each core
transforms only its own nodes: x@[W1|A1] resp. h@[W2|A2], PSUM slices on a
512-f32 pitch so no matmul output crosses a 2KB PSUM bank) and AllGather'ed
into rank-major gather tables — the halo exchange of transformed src node
features.  tab1 rows [feat(d-major) | el | er] bf16 (768B rows for
dma_gather), tab2 rows 512B.  Because both tables share the rank-major id
space, one meta/midx tensor pair serves both edge phases.  Edge phase per
block: one int8 meta DMA carries the pre-built fp8 scatter one-hot (PE
accepts fp8 lhsT with bf16 rhs, so no per-chunk DVE is_equal builds); the
gather one-hot is derived on-chip as oh^T via plain PE matmuls against the
identity (halving the meta traffic), with the PSUM round-trip copies
alternating between DVE and ACT; a small separate DMA carries the int16
gather indices; multi-packet dma_gather fetches src rows into one fused
lo+hi tile; PE matmuls broadcast er to slots and scatter-add messages +
exp-sums into PSUM.  Features use a d-major (d,h) layout so the
attention-weight broadcast is along a non-innermost axis, keeping the DVE
2x mode for the message multiply; leaky-relu runs on DVE (the Lrelu ACT
table lives in a different func set than Exp and would reload every
block), exp/relu on ACT.  Per-dst softmax normalization happens after the
reduction (max-subtraction skipped; |e| is O(1)).

Host precomputes (numpy, index/layout-only plus weight repacking): slot
maps, fp8 one-hots, wrapped int16 gather indices, d-major-permuted
[W | W@al | W@ar] rhs blocks, bias rows, and the bf16 x transpose.
"""

import os

import numpy as np

import concourse.bacc as bacc
import concourse.bass as bass
import concourse.mybir as mybir
import concourse.tile as tile
from concourse.masks import make_identity

F32 = mybir.dt.float32
BF16 = mybir.dt.bfloat16
I32 = mybir.dt.int32
I16 = mybir.dt.int16
I8 = mybir.dt.int8
FP8 = mybir.dt.float8e4
AF = mybir.ActivationFunctionType
OP = mybir.AluOpType

P = 128
HALF_LIMIT = 32768  # int16 gather index limit
ONE_FP8 = 0x38      # float8e4m3 encoding of 1.0


class GATCfg:
    def __init__(self, N=50000, C=8, IN=128, HID=32, HEADS=8, OUT=16, NEG=0.2):
        self.N, self.C, self.IN = N, C, IN
        self.HID, self.HEADS, self.OUT, self.NEG = HID, HEADS, OUT, NEG
        self.F1 = HEADS * HID        # 256
        self.F2 = HEADS * OUT        # 128
        self.FP8T1 = os.environ.get("GAT_FP8T1", "0") == "1"
        # tab1 row in BYTES: [feat | el]; fp8 feat: 256+16 -> 512B rows,
        # bf16 feat: 512+16 -> 768B rows (dma_gather needs 256B multiples)
        self.ROW1B = 512 if self.FP8T1 else 768
        self.T1USED = (256 if self.FP8T1 else 512) + 16
        self.ROW2 = 256              # bf16 elems; 512B rows (used: 128+8+8)
        self.Nloc = (N + C - 1) // C
        self.NB = (self.Nloc + P - 1) // P
        self.Nlp = self.NB * P
        self.NP1 = ((N + 511) // 512) * 512
        self.NP2 = C * self.Nlp
        NPmax = max(self.NP1, self.NP2)
        h = (NPmax // 2 + P - 1) // P * P
        self.HALF = (min(HALF_LIMIT, max(h, NPmax - HALF_LIMIT))
                     if NPmax > HALF_LIMIT else NPmax)
        self.HALF = max(self.HALF, NPmax - HALF_LIMIT)
        self.SLO = 0
        self.SHI = 0

    @property
    def NCt(self):
        return (self.SLO + self.SHI) // P

    def meta_cols(self):
        return self.NCt * P

    def midx_cols(self):
        return 2 * (self.SLO // 16) + 2 * (self.SHI // 16)


def _wrap16(vals_slots, S):
    """[S] slot-ordered ints -> [128, S//16] 16-wrapped, replicated 8x."""
    a = vals_slots.reshape(S // 16, 16)
    out = np.zeros((128, S // 16), np.int16)
    blkcols = a.T.astype(np.int16)  # [16, S//16]
    for r in range(8):
        out[r * 16:(r + 1) * 16, :] = blkcols
    return out


def prep_indices(src, dst, cfg):
    """Host index-only preprocessing: slot maps, gather indices, fp8 one-hots."""
    C, Nloc, NB = cfg.C, cfg.Nloc, cfg.NB
    src = np.asarray(src).astype(np.int64)
    dst = np.asarray(dst).astype(np.int64)
    core = dst // Nloc
    dloc = dst - core * Nloc
    blk = dloc // P
    dblk = dloc - blk * P
    key = core * NB + blk
    order = np.argsort(key, kind="stable")
    counts = np.bincount(key, minlength=C * NB)
    starts = np.zeros(C * NB + 1, np.int64)
    np.cumsum(counts, out=starts[1:])

    r_of = src // Nloc
    srcp = r_of * cfg.Nlp + (src - r_of * Nloc)   # layer-2 rank-major id

    HALFc = cfg.HALF

    def lohi_max(ids):
        lo_max = hi_max = 0
        for k in range(C * NB):
            e = order[starts[k]:starts[k + 1]]
            n_lo = int((ids[e] < HALFc).sum())
            lo_max = max(lo_max, n_lo)
            hi_max = max(hi_max, e.size - n_lo)
        return lo_max, hi_max

    lo1, hi1 = lohi_max(srcp)
    SLO = ((max(lo1, 1) + P - 1) // P) * P
    SHI = ((max(hi1, 1) + P - 1) // P) * P
    cfg.SLO, cfg.SHI = SLO, SHI
    NCt = cfg.NCt
    CL, CH = SLO // 16, SHI // 16
    MC = cfg.meta_cols()

    out = {}
    MI = 2 * CL + 2 * CH
    for layer, ids in ((1, srcp),):
        meta = np.zeros((C, NB, 128, MC), np.uint8)
        midx = np.zeros((C, NB, 128, MI), np.uint8)
        for c in range(C):
            for b in range(NB):
                k = c * NB + b
                e = order[starts[k]:starts[k + 1]]
                v = ids[e]
                m = v < HALFc
                elo, ehi = e[m], e[~m]
                ilo = np.zeros(SLO, np.int64)
                ihi = np.zeros(SHI, np.int64)
                ilo[:elo.size] = v[m]
                ihi[:ehi.size] = v[~m] - HALFc
                # slot ids: lo at [0,nlo), hi at [SLO, SLO+nhi)
                sl = np.concatenate([np.arange(elo.size),
                                     SLO + np.arange(ehi.size)])
                dv = dblk[np.concatenate([elo, ehi])]
                oh = np.zeros((128, NCt * P), np.uint8)
                # oh[p, c*128 + m] = (m == d(slot c*128+p))
                oh[sl % P, (sl // P) * P + dv] = ONE_FP8
                wl = _wrap16(ilo, SLO).view(np.uint8).reshape(128, 2 * CL)
                wh = _wrap16(ihi, SHI).view(np.uint8).reshape(128, 2 * CH)
                meta[c, b] = oh
                midx[c, b] = np.concatenate([wl, wh], axis=1)
        out[f"meta{layer}"] = np.ascontiguousarray(
            meta.transpose(0, 2, 1, 3).reshape(C, 128, NB * MC))
        out[f"midx{layer}"] = np.ascontiguousarray(
            midx.transpose(0, 2, 1, 3).reshape(C, 128, NB * MI))
    return out


def _dperm(H, D):
    """permutation p with p[d*H+h] = h*D+d (d-major ordering)."""
    idx = np.arange(H * D).reshape(H, D).T.reshape(-1)
    return idx


def host_inputs(inputs, cfg, idx):
    import ml_dtypes
    BF = ml_dtypes.bfloat16
    H, D1, D2 = cfg.HEADS, cfg.HID, cfg.OUT
    F1, F2 = cfg.F1, cfg.F2
    p1 = _dperm(H, D1)   # F1 perm
    p2 = _dperm(H, D2)   # F2 perm

    x = np.asarray(inputs["x"], np.float32)
    xT = np.zeros((cfg.IN, cfg.N), np.float32)
    xT[:, :cfg.N] = np.ascontiguousarray(x.T)

    W1 = np.asarray(inputs["W1"], np.float32)
    W2 = np.asarray(inputs["W2"], np.float32)
    al1 = np.asarray(inputs["al1"], np.float32)
    ar1 = np.asarray(inputs["ar1"], np.float32)
    al2 = np.asarray(inputs["al2"], np.float32)
    ar2 = np.asarray(inputs["ar2"], np.float32)
    b1 = np.asarray(inputs["b1"], np.float32)
    b2 = np.asarray(inputs["b2"], np.float32)

    # el/er projection matrices [IN, 16]: col h = W1[:, head h] @ al1[h]
    A1 = np.zeros((cfg.IN, 16), np.float32)
    for h in range(H):
        A1[:, h] = W1[:, h * D1:(h + 1) * D1] @ al1[h]
        A1[:, 8 + h] = W1[:, h * D1:(h + 1) * D1] @ ar1[h]
    RHS1 = np.concatenate([W1[:, p1], A1], axis=1)          # [IN, 272]

    # layer2: rows of W2 permuted to d-major F1; cols to d-major F2
    W2p = W2[p1][:, p2]                                      # [256, 128]
    A2 = np.zeros((F1, 16), np.float32)
    for h in range(H):
        A2[:, h] = W2[:, h * D2:(h + 1) * D2] @ al2[h]
        A2[:, 8 + h] = W2[:, h * D2:(h + 1) * D2] @ ar2[h]
    A2 = A2[p1]
    RHS2f = np.concatenate([W2p, A2], axis=1)                # [256, 144]
    # pack as [128, 2, 144] (partition = K within half)
    RHS2 = np.ascontiguousarray(
        RHS2f.reshape(2, 128, 144).transpose(1, 0, 2))

    B1 = np.broadcast_to(b1[p1][None, :], (128, F1)).astype(np.float32)
    b2m = b2.reshape(H, D2).mean(axis=0)                     # [16]
    B2M = np.broadcast_to(b2m[None, :], (128, D2)).astype(np.float32)

    in_maps = []
    for c in range(cfg.C):
        xTl = np.zeros((cfg.IN, cfg.Nlp), np.float32)
        lo = c * cfg.Nloc
        hi = min(cfg.N, lo + cfg.Nloc)
        xTl[:, :hi - lo] = xT[:, lo:hi]
        in_maps.append({
            "xTl": xTl.astype(BF),
            "RHS1": RHS1.astype(BF), "RHS2": RHS2.astype(BF),
            "B1": np.ascontiguousarray(B1),
            "B2M": np.ascontiguousarray(B2M),
            "meta1": idx["meta1"][c].view(np.int8),
            "midx1": idx["midx1"][c].view(np.int8),
        })
    return in_maps


def build_module(cfg, dbg=False, skip_cc=False):
    nc = bacc.Bacc("TRN2", target_bir_lowering=False, debug=False,
                   num_devices=cfg.C)
    C, NB, Nlp = cfg.C, cfg.NB, cfg.Nlp
    F1, F2, ROW2 = cfg.F1, cfg.F2, cfg.ROW2
    ROW1E = cfg.ROW1B // 2       # tab1 row in bf16 elems (bf16 mode only)
    assert not cfg.FP8T1
    SLO, SHI, NCt = cfg.SLO, cfg.SHI, cfg.NCt
    NLOC, NHIC = SLO // P, SHI // P
    CL, CH = SLO // 16, SHI // 16
    MC = cfg.meta_cols()
    MI = cfg.midx_cols()
    OUTW = cfg.OUT
    NT1 = cfg.NP1 // P            # 392 layer-1 table tiles
    TB1 = 8                       # tiles per L1 table DMA group
    NG1 = NT1 // TB1              # 49
    TL2 = 7                       # tiles per L2 table load
    NG2 = NB // TL2               # 7

    d_xTl = nc.dram_tensor("xTl", [cfg.IN, Nlp], BF16, kind="ExternalInput")
    d_RHS1 = nc.dram_tensor("RHS1", [cfg.IN, F1 + 16], BF16,
                            kind="ExternalInput")
    d_RHS2 = nc.dram_tensor("RHS2", [P, 2, F2 + 16], BF16,
                            kind="ExternalInput")
    d_B1 = nc.dram_tensor("B1", [P, F1], F32, kind="ExternalInput")
    d_B2M = nc.dram_tensor("B2M", [P, OUTW], F32, kind="ExternalInput")
    d_meta1 = nc.dram_tensor("meta1", [P, NB * MC], I8, kind="ExternalInput")
    d_midx1 = nc.dram_tensor("midx1", [P, NB * MI], I8, kind="ExternalInput")
    d_out = nc.dram_tensor("out", [NB, P, OUTW], F32, kind="ExternalOutput")

    d_t1l = nc.dram_tensor("t1l", [NB, P, ROW1E], BF16, kind="Internal")
    d_tab1 = nc.dram_tensor("tab1", [C, NB, P, ROW1E], BF16, kind="Internal",
                            addr_space="Shared" if C > 4 else "Local")
    d_t2l = nc.dram_tensor("t2l", [NB, P, ROW2], BF16, kind="Internal")
    d_tab2 = nc.dram_tensor("tab2", [C, NB, P, ROW2], BF16, kind="Internal",
                            addr_space="Shared" if C > 4 else "Local")
    d_hT = nc.dram_tensor("hT", [P, NB, 2, P], BF16, kind="Internal")

    tab1_flat = d_tab1.rearrange("r t p c -> (r t p) c")
    tab2_flat = d_tab2.rearrange("r t p c -> (r t p) c")

    with tile.TileContext(nc) as tc:
        with (
            tc.tile_pool(name="const", bufs=1) as cpool,
            tc.tile_pool(name="work", bufs=6) as wpool,
            tc.tile_pool(name="gath", bufs=2) as gpool,
            tc.tile_pool(name="stage", bufs=4) as spool,
            tc.tile_pool(name="idx", bufs=8) as ipool,
            tc.tile_pool(name="oht", bufs=4) as opool,
        ):
            # ---------------- constants ----------------
            ident = cpool.tile([P, P], BF16)
            make_identity(nc, ident[:])
            rhs1 = cpool.tile([P, F1 + 16], BF16)
            nc.sync.dma_start(rhs1[:], d_RHS1[:, :])
            rhs2 = cpool.tile([P, 2, F2 + 16], BF16)
            nc.sync.dma_start(rhs2[:], d_RHS2[:, :, :])
            b1row = cpool.tile([P, F1], F32)
            nc.sync.dma_start(b1row[:], d_B1[:, :])
            b2mean = cpool.tile([P, OUTW], F32)
            nc.sync.dma_start(b2mean[:], d_B2M[:, :])

            # ------- local layer-1 table (rows [feat|el|er]) + allgather ---
            with tc.tile_pool(name="t1ps", bufs=2, space="PSUM") as t1ps:
                for g in range(NG2):
                    xl = wpool.tile([P, TL2 * P], BF16, tag="xl")
                    nc.sync.dma_start(
                        xl[:], d_xTl[:, g * TL2 * P:(g + 1) * TL2 * P])
                    stg = spool.tile([P, TL2, F1 + 16], BF16, tag="stg1")
                    for q0, qn in ((0, 2), (2, 2), (4, 2), (6, 1)):
                        # 512-f32 slice pitch keeps each matmul output inside
                        # a single 2KB PSUM bank (outputs must not cross one)
                        ps = t1ps.tile([P, 2, 512], F32, tag="t1")
                        for m in range(qn):
                            nc.tensor.matmul(
                                ps[:, m, 0:F1 + 16],
                                lhsT=xl[:, (q0 + m) * P:(q0 + m + 1) * P],
                                rhs=rhs1[:], start=True, stop=True)
                        (nc.vector.tensor_copy if q0 % 4 == 0
                         else nc.scalar.copy)(
                            stg[:, q0:q0 + qn, :],
                            ps[:, 0:qn, 0:F1 + 16])
                    nc.sync.dma_start(
                        d_t1l[g * TL2:(g + 1) * TL2, :, 0:F1 + 16]
                        .rearrange("t p c -> p t c"), stg[:])

            if C > 1 and not skip_cc:
                nc.gpsimd.collective_compute(
                    "AllGather", OP.bypass,
                    replica_groups=[list(range(C))],
                    ins=[d_t1l[:, :, :]],
                    outs=[d_tab1[:, :, :, :]],
                )

            # ---------------- edge phase (shared) ----------------
            def edge_phase(layer, pspool, tps, tpp):
                F = F1 if layer == 1 else F2
                ROW = ROW1E if layer == 1 else ROW2
                tab = tab1_flat if layer == 1 else tab2_flat
                d_meta, d_midx = d_meta1, d_midx1
                PIECE = 768
                single_packet = "mp" not in os.environ.get("GAT_OPT", "mp")
                for b in range(NB):
                    midx = ipool.tile([P, MI], I8, tag="midx")
                    nc.sync.dma_start(midx[:], d_midx[:, b * MI:(b + 1) * MI])
                    meta = wpool.tile([P, MC], I8, tag="meta")
                    h2 = MC // 2
                    nc.sync.dma_start(meta[:, 0:h2],
                                      d_meta[:, b * MC:b * MC + h2])
                    nc.sync.dma_start(meta[:, h2:MC],
                                      d_meta[:, b * MC + h2:(b + 1) * MC])
                    il = midx[:, 0:2 * CL].bitcast(I16)
                    ih = midx[:, 2 * CL:MI].bitcast(I16)
                    erblk = ipool.tile([P, 8], BF16, tag="erblk")
                    if layer == 1:
                        nc.sync.dma_start(erblk[:],
                                          d_t1l[b, :, F1 + 8:F1 + 16])
                    else:
                        nc.sync.dma_start(erblk[:],
                                          d_t2l[b, :, F2 + 8:F2 + 16])
                    G = gpool.tile([P, NCt, ROW], BF16, tag=f"G{layer}")
                    Gfeat = G[:, :, 0:F]
                    Gel = G[:, :, F:F + 8]
                    if single_packet:
                        pieces = [(s0, min(PIECE, SLO - s0), 0)
                                  for s0 in range(0, SLO, PIECE)]
                        pieces += [(s0, min(PIECE, SHI - s0), 1)
                                   for s0 in range(0, SHI, PIECE)]
                    else:
                        pieces = [(0, SLO, 0), (0, SHI, 1)]
                    for s0, n, is_hi in pieces:
                        idxs = ih if is_hi else il
                        base = SLO if is_hi else 0
                        src_ap = tab[cfg.HALF:, :] if is_hi else tab[:, :]
                        nc.gpsimd.dma_gather(
                            out_ap=G[:, (base + s0) // P:(base + s0 + n) // P, :],
                            in_ap=src_ap,
                            idxs_ap=idxs[:, s0 // 16:(s0 + n) // 16],
                            num_idxs=n, num_idxs_reg=n, elem_size=ROW,
                            single_packet=single_packet)

                    # derive the gather one-hot ohT = oh^T on PE via a
                    # plain matmul against the identity (halves the meta DMA)
                    ohT = opool.tile([P, NCt, P], BF16, tag="ohT")
                    for i, g0 in enumerate(range(0, NCt, 4)):
                        gn = min(4, NCt - g0)
                        tp_ = tpp.tile([P, 4, P], F32, tag="ohTp")
                        for c in range(gn):
                            nc.tensor.matmul(
                                tp_[:, c, :],
                                lhsT=meta[:, (g0 + c) * P:(g0 + c + 1) * P]
                                .bitcast(FP8),
                                rhs=ident[:], start=True, stop=True)
                        (nc.scalar.copy if i % 2 == 0
                         else nc.vector.tensor_copy)(
                            ohT[:, g0:g0 + gn, :], tp_[:, 0:gn, :])
                    # er broadcast to slots
                    erps = tps.tile([P, NCt, 8], F32, tag="erps")
                    for c in range(NCt):
                        nc.tensor.matmul(
                            erps[:, c, :],
                            lhsT=ohT[:, c, :],
                            rhs=erblk[:], start=True, stop=True)
                    e_t = wpool.tile([P, NCt, 8], BF16, tag="e_t")
                    nc.vector.tensor_tensor(out=e_t[:], in0=Gel,
                                            in1=erps[:], op=OP.add)
                    # leaky_relu on DVE: es = max(e, 0.2*e)  (the Lrelu ACT
                    # table lives in a different func set than Exp; switching
                    # would reload the 1283ns act table every block)
                    ea = wpool.tile([P, NCt, 8], BF16, tag="ea")
                    nc.vector.tensor_scalar(ea[:], e_t[:], cfg.NEG, None,
                                            op0=OP.mult)
                    es = wpool.tile([P, NCt, 8], BF16, tag="es")
                    nc.vector.tensor_tensor(out=es[:], in0=e_t[:], in1=ea[:],
                                            op=OP.max)
                    MSG = gpool.tile([P, NCt, F + 8], BF16, tag=f"MSG{layer}")
                    nc.scalar.activation(MSG[:, :, F:F + 8], es[:], AF.Exp)
                    nc.vector.tensor_tensor(
                        out=MSG[:, :, 0:F].rearrange(
                            "p c (d h) -> p c d h", h=8),
                        in0=Gfeat.rearrange("p c (d h) -> p c d h", h=8),
                        in1=MSG[:, :, F:F + 8].rearrange(
                            "p c (one h) -> p c one h", one=1)
                        .to_broadcast([P, NCt, F // 8, 8]),
                        op=OP.mult)
                    ps = pspool.tile([P, F + 8], F32, tag="eps")
                    for c in range(NCt):
                        nc.tensor.matmul(
                            ps[:],
                            lhsT=meta[:, c * P:(c + 1) * P].bitcast(FP8),
                            rhs=MSG[:, c, :],
                            start=(c == 0), stop=(c == NCt - 1))
                    esum = wpool.tile([P, 8], F32, tag="esum")
                    nc.vector.tensor_scalar(esum[:], ps[:, F:F + 8], 1e-30,
                                            None, op0=OP.max)
                    inv = wpool.tile([P, 8], F32, tag="inv")
                    nc.vector.reciprocal(inv[:], esum[:])
                    yield b, ps, inv

            # ---------------- layer-1 edges + hT + local tab2 ----------------
            with tc.tile_pool(name="e1ps", bufs=2, space="PSUM") as e1ps, \
                 tc.tile_pool(name="tps", bufs=2, space="PSUM") as tps, \
                 tc.tile_pool(name="tp1", bufs=2, space="PSUM") as tpp1:
                for b, ps, inv in edge_phase(1, e1ps, tps, tpp1):
                    z = wpool.tile([P, F1], F32, tag="z")
                    nc.vector.tensor_tensor(
                        out=z[:].rearrange("p (d h) -> p d h", h=8),
                        in0=ps[:, 0:F1].rearrange("p (d h) -> p d h", h=8),
                        in1=inv[:].rearrange("p (one h) -> p one h", one=1)
                        .to_broadcast([P, F1 // 8, 8]),
                        op=OP.mult)
                    nc.vector.tensor_add(z[:], z[:], b1row[:])
                    # elu(z) = relu(z) + exp(min(z,0)) - 1
                    # min(z,0) = -relu(-z); both relu and exp run on ACT
                    zmn = wpool.tile([P, F1], BF16, tag="zmn")
                    nc.scalar.activation(zmn[:], z[:], AF.Relu, scale=-1.0)
                    zp = wpool.tile([P, F1], BF16, tag="zp")
                    nc.scalar.activation(zp[:], z[:], AF.Relu)
                    q_ = wpool.tile([P, F1], BF16, tag="q_")
                    nc.scalar.activation(q_[:], zmn[:], AF.Exp, scale=-1.0)
                    hb = wpool.tile([P, F1], BF16, tag="hb")
                    nc.vector.affine_then_add(hb[:], q_[:], zp[:], 1.0, -1.0)
                    pst = tps.tile([P, 2, P], BF16, tag="pst")
                    for q in range(2):
                        nc.tensor.transpose(pst[:, q, :],
                                            hb[:, q * P:(q + 1) * P], ident[:])
                    htp = wpool.tile([P, 2, P], BF16, tag="htp")
                    nc.vector.tensor_copy(htp[:], pst[:])
                    nc.sync.dma_start(d_hT[:, b, :, :], htp[:])

            # ---------------- local layer-2 table + allgather ----------------
            with tc.tile_pool(name="t2ps", bufs=2, space="PSUM") as t2ps:
                for g in range(NG2):
                    ht = wpool.tile([P, TL2, 2, P], BF16, tag="ht")
                    nc.sync.dma_start(ht[:], d_hT[:, g * TL2:(g + 1) * TL2,
                                                  :, :])
                    # 256-f32 slice pitch: every 144-wide output stays inside
                    # one 2KB PSUM bank
                    ps2 = t2ps.tile([P, TL2, 256], F32, tag="t2")
                    for m in range(TL2):
                        nc.tensor.matmul(ps2[:, m, 0:F2 + 16],
                                         lhsT=ht[:, m, 0, :],
                                         rhs=rhs2[:, 0, :],
                                         start=True, stop=False)
                        nc.tensor.matmul(ps2[:, m, 0:F2 + 16],
                                         lhsT=ht[:, m, 1, :],
                                         rhs=rhs2[:, 1, :],
                                         start=False, stop=True)
                    st2 = spool.tile([P, TL2, F2 + 16], BF16, tag="stg2")
                    (nc.vector.tensor_copy if g % 2 == 0 else nc.scalar.copy)(
                        st2[:], ps2[:, :, 0:F2 + 16])
                    nc.sync.dma_start(
                        d_t2l[g * TL2:(g + 1) * TL2, :, 0:F2 + 16]
                        .rearrange("t p c -> p t c"), st2[:])

            if C > 1 and not skip_cc:
                nc.gpsimd.collective_compute(
                    "AllGather", OP.bypass,
                    replica_groups=[list(range(C))],
                    ins=[d_t2l[:, :, :]],
                    outs=[d_tab2[:, :, :, :]],
                )

            # ---------------- layer-2 edges + output ----------------
            with tc.tile_pool(name="e2ps", bufs=2, space="PSUM") as e2ps, \
                 tc.tile_pool(name="tps2", bufs=3, space="PSUM") as tps2, \
                 tc.tile_pool(name="tp2", bufs=3, space="PSUM") as tpp2:
                ostage = None
                for b, ps, inv in edge_phase(2, e2ps, tps2, tpp2):
                    if b % 8 == 0:
                        ostage = spool.tile([P, 8, OUTW], F32, tag="ostage")
                    inv8 = wpool.tile([P, 8], F32, tag="inv8")
                    nc.vector.tensor_scalar(inv8[:], inv[:], 0.125, None,
                                            op0=OP.mult)
                    w_ = wpool.tile([P, OUTW, 8], F32, tag="w_")
                    nc.vector.tensor_tensor(
                        out=w_[:],
                        in0=ps[:, 0:F2].rearrange("p (d h) -> p d h", h=8),
                        in1=inv8[:].rearrange("p (one h) -> p one h", one=1)
                        .to_broadcast([P, OUTW, 8]),
                        op=OP.mult)
                    s1 = wpool.tile([P, OUTW, 4], F32, tag="s1")
                    nc.vector.tensor_add(s1[:], w_[:, :, 0:4], w_[:, :, 4:8])
                    s2 = wpool.tile([P, OUTW, 2], F32, tag="s2")
                    nc.vector.tensor_add(s2[:], s1[:, :, 0:2], s1[:, :, 2:4])
                    s3 = wpool.tile([P, OUTW], F32, tag="s3")
                    nc.vector.tensor_add(s3[:], s2[:, :, 0], s2[:, :, 1])
                    nc.vector.tensor_add(ostage[:, b % 8, :], s3[:],
                                         b2mean[:])
                    if b % 8 == 7 or b == NB - 1:
                        b0 = (b // 8) * 8
                        nt = b - b0 + 1
                        nc.sync.dma_start(
                            d_out[b0:b0 + nt, :, :]
                            .rearrange("t p c -> p t c"),
                            ostage[:, 0:nt, :])

            if dbg:
                for nm, src_t in [("dbg_t1l", d_t1l), ("dbg_tab1", d_tab1),
                                  ("dbg_hT", d_hT), ("dbg_t2l", d_t2l),
                                  ("dbg_tab2", d_tab2)]:
                    dd = nc.dram_tensor(nm, list(src_t.shape), BF16,
                                        kind="ExternalOutput")
                    sl = tuple(slice(None) for _ in src_t.shape)
                    nc.sync.dma_start(dd[sl], src_t[sl])

    nc.compile()
    return nc


# ----------------------------------------------------------------------------
_CACHE = {}


def get_built(src, dst, C=8, cfg=None):
    key = (hash(src.tobytes()), hash(dst.tobytes()), C)
    if key not in _CACHE:
        if cfg is None:
            cfg = GATCfg(C=C)
        idx = prep_indices(src, dst, cfg)
        nc = build_module(cfg)
        _CACHE[key] = (cfg, idx, nc)
    return _CACHE[key]


_EXECC = {}


def _get_exec(key, nc, n_cores):
    """Persistent jit(shard_map(bass_exec)) so repeated kernel() calls skip
    retracing/recompiling."""
    if key in _EXECC:
        return _EXECC[key]
    import jax
    from jax.experimental.shard_map import shard_map
    from jax.sharding import Mesh, NamedSharding, PartitionSpec
    from concourse import bass2jax
    bass2jax.install_neuronx_cc_hook()
    partition_name = (nc.partition_id_tensor.name
                      if nc.partition_id_tensor else None)
    in_names, out_names, out_avals, zero_shapes = [], [], [], []
    for alloc in nc.m.functions[0].allocations:
        if not isinstance(alloc, mybir.MemoryLocationSet):
            continue
        name = alloc.memorylocations[0].name
        if alloc.kind == "ExternalInput":
            if name != partition_name:
                in_names.append(name)
        elif alloc.kind == "ExternalOutput":
            out_names.append(name)
            shape = tuple(alloc.tensor_shape)
            dtype = mybir.dt.np(alloc.dtype)
            out_avals.append(jax.core.ShapedArray(shape, dtype))
            zero_shapes.append((shape, dtype))
    n_params = len(in_names)
    in_names_all = list(in_names) + out_names + (
        [partition_name] if partition_name else [])

    def _body(*args):
        ops = list(args)
        if partition_name:
            ops.append(bass2jax.partition_id_tensor())
        outs = bass2jax._bass_exec_p.bind(
            *ops, out_avals=tuple(out_avals), in_names=tuple(in_names_all),
            out_names=tuple(out_names), lowering_input_output_aliases=(),
            sim_require_finite=True, sim_require_nnan=True, nc=nc)
        return tuple(outs)

    devices = jax.devices()[:n_cores]
    mesh = Mesh(np.asarray(devices), ("core",))
    nout = len(out_names)
    f = jax.jit(shard_map(
        _body, mesh=mesh,
        in_specs=(PartitionSpec("core"),) * (n_params + nout),
        out_specs=(PartitionSpec("core"),) * nout, check_rep=False),
        keep_unused=True)
    sh = NamedSharding(mesh, PartitionSpec("core"))
    ent = dict(f=f, in_names=in_names, out_names=out_names,
               zero_shapes=zero_shapes, sh=sh, argcache=None)
    _EXECC[key] = ent
    return ent


def kernel(**inputs) -> np.ndarray:
    import jax
    src = np.asarray(inputs["src"], np.int32)
    dst = np.asarray(inputs["dst"], np.int32)
    x = np.asarray(inputs["x"])
    base = GATCfg(N=int(x.shape[0]), C=8, IN=int(x.shape[1]))
    cfg, idx, nc = get_built(src, dst, C=8, cfg=base)
    in_maps = host_inputs(inputs, cfg, idx)
    key = (hash(src.tobytes()), hash(dst.tobytes()), cfg.C)
    ent = _get_exec(key, nc, cfg.C)
    C = cfg.C
    concat_in = [np.ascontiguousarray(
        np.concatenate([in_maps[c][nm] for c in range(C)], axis=0))
        for nm in ent["in_names"]]
    hashes = tuple(hash(a.tobytes()) for a in concat_in)
    if ent["argcache"] is None or ent["argcache"][0] != hashes:
        zeros = [np.zeros((C * sh0[0], *sh0[1:]), dt)
                 for sh0, dt in ent["zero_shapes"]]
        args = [jax.device_put(a, ent["sh"]) for a in concat_in + zeros]
        ent["argcache"] = (hashes, args)
    args = ent["argcache"][1]
    outs = ent["f"](*args)
    jax.block_until_ready(outs)
    oi = ent["out_names"].index("out")
    out = np.asarray(outs[oi]).reshape(C, cfg.Nlp, cfg.OUT)
    full = out[:, :cfg.Nloc, :].reshape(-1, cfg.OUT)[:cfg.N]
    return np.ascontiguousarray(full.astype(np.float32))
